# revision 8
# baseline (speedup 1.0000x reference)
"""Multi-head attention TRN2 Bass kernel.

Problem: B=2, S=2048, E=1024, H=16, Dh=64; per-head QKV projection weights,
unmasked softmax(Q K^T / sqrt(Dh)) @ V, concat heads, out-projection.

Sharding: 8 cores = 2 batches x 4 head-groups (4 heads each). Each core
computes its batch/head-group's attention and a partial out-projection
(the rows of Wo belonging to its heads); the host sums the 4 partials per
batch and adds bo.

Numerics: the attention logits here have std ~1200, so softmax is
near-one-hot and argmax flips dominate the error. The Q/K projection and
the scores matmul therefore run in fp32. The row-max used for the softmax
shift is computed from a bf16 scores pass (softmax is invariant to the
shift as long as it is within ~80 of the true max; we add a safety margin).
The max-subtraction is folded into the transposed-scores matmul via an
augmented contraction row, and the softmax denominator falls out of an
extra ones-column on V.
"""

import numpy as np

import concourse.bacc as bacc
import concourse.bass as bass
import concourse.mybir as mybir
import concourse.tile as tile
from concourse import masks
from concourse.bass_utils import run_bass_kernel_spmd

F32 = mybir.dt.float32
BF16 = mybir.dt.bfloat16
AX = mybir.AxisListType
AF = mybir.ActivationFunctionType
ALU = mybir.AluOpType

B, S, E, H, DH = 2, 2048, 1024, 16, 64
NCORES = 8
HPC = 4          # heads per core
NPAIR = 2        # head pairs per core
ET = E // 128    # 8 e-tiles
ST = S // 128    # 16 s-tiles
IB = S // 512    # 4 i-blocks
JT = S // 128    # 16 j-tiles
MARGIN = 32.0    # safety margin for the approximate row max

# dtype of att^T / V-hat for the att@V matmul (flip to BF16 for speed,
# costs ~2.7e-3 absmax relative error)
ATT_DTYPE = F32
SCORES_A_BF16 = True


def build_bass():
    nc = bacc.Bacc("TRN2", target_bir_lowering=False, debug=False,
                   num_devices=NCORES)
    x_q = nc.dram_tensor("x_q", [S, E], F32, kind="ExternalInput")
    x_k = nc.dram_tensor("x_k", [S, E], F32, kind="ExternalInput")
    x_v = nc.dram_tensor("x_v", [S, E], F32, kind="ExternalInput")
    wq = nc.dram_tensor("wq", [NPAIR, ET, 128, 128], F32, kind="ExternalInput")
    wk = nc.dram_tensor("wk", [NPAIR, ET, 128, 128], F32, kind="ExternalInput")
    wv = nc.dram_tensor("wv", [NPAIR, ET, 128, 128], F32, kind="ExternalInput")
    bqs = nc.dram_tensor("bqs", [128, NPAIR], F32, kind="ExternalInput")
    bks = nc.dram_tensor("bks", [128, NPAIR], F32, kind="ExternalInput")
    bvb = nc.dram_tensor("bvb", [128, NPAIR, 128], F32, kind="ExternalInput")
    wo = nc.dram_tensor("wo", [NPAIR, 128, E], F32, kind="ExternalInput")
    out_p = nc.dram_tensor("out_p", [S, E], F32, kind="ExternalOutput")

    with tile.TileContext(nc) as tc:
        with (
            tc.tile_pool(name="const", bufs=1) as const_pool,
            tc.tile_pool(name="persist", bufs=1) as persist,
        ):
            ident = const_pool.tile([128, 128], F32, name="ident")
            masks.make_identity(nc, ident[:])
            marg = const_pool.tile([128, 1], F32, name="marg")
            nc.gpsimd.memset(marg[:], -MARGIN)

            bqs_sb = const_pool.tile([128, NPAIR], F32, name="bqs")
            nc.sync.dma_start(bqs_sb[:], bqs[:])
            bks_sb = const_pool.tile([128, NPAIR], F32, name="bks")
            nc.sync.dma_start(bks_sb[:], bks[:])
            bvb_sb = const_pool.tile([128, NPAIR, 128], F32, name="bvb")
            nc.sync.dma_start(bvb_sb[:], bvb[:])
            wo_sb = const_pool.tile([128, NPAIR, E], F32, name="wo")
            nc.sync.dma_start(wo_sb[:], wo.rearrange("c p e -> p c e"))

            # persistent per-head tensors
            qh = [persist.tile([65, S], F32, name=f"qh{h}") for h in range(HPC)]
            kh = [persist.tile([65, S], F32, name=f"kh{h}") for h in range(HPC)]
            vh = [persist.tile([128, JT, 65], ATT_DTYPE, name=f"vh{h}")
                  for h in range(HPC)]
            if SCORES_A_BF16:
                qa = [persist.tile([64, S], BF16, name=f"qa{h}") for h in range(HPC)]
                ka = [persist.tile([64, S], BF16, name=f"ka{h}") for h in range(HPC)]
            ctxT = [persist.tile([128, S], F32, name=f"ctxT{c}") for c in range(2)]

            for h in range(HPC):
                nc.gpsimd.memset(kh[h][64:65, :], 1.0)
                nc.gpsimd.memset(vh[h][:, :, 64:65], 1.0)

            # ---- phase 1: load, transpose, project ----
            with (
                tc.tile_pool(name="stage", bufs=3) as stage_pool,
                tc.tile_pool(name="xt", bufs=3) as xt_pool,
                tc.tile_pool(name="wght", bufs=1) as w_pool,
                tc.tile_pool(name="ps_tin", bufs=3, space="PSUM") as ps_tin,
                tc.tile_pool(name="ps_proj", bufs=4, space="PSUM") as ps_proj,
            ):
                for x_dram, w_dram, which in ((x_q, wq, "q"), (x_k, wk, "k"),
                                              (x_v, wv, "v")):
                    w_sb = w_pool.tile([128, NPAIR, ET, 128], F32, name="w_in")
                    nc.sync.dma_start(w_sb[:], w_dram.rearrange("p t e d -> e p t d"))
                    for st in range(ST):
                        sl = bass.ts(st, 128)
                        stage = stage_pool.tile([128, E], F32, name="stage")
                        nc.sync.dma_start(stage[:], x_dram[sl, :])
                        xt = xt_pool.tile([128, ET, 128], F32, name="xt")
                        for et in range(ET):
                            pst = ps_tin.tile([128, 128], F32, name="pst")
                            nc.tensor.transpose(pst[:], stage[:, bass.ts(et, 128)],
                                                ident[:])
                            nc.vector.tensor_copy(xt[:, et, :], pst[:])
                        for p in range(NPAIR):
                            psp = ps_proj.tile([128, 128], F32, name="psp")
                            for et in range(ET):
                                if which == "v":
                                    lhsT, rhs = xt[:, et, :], w_sb[:, p, et, :]
                                else:
                                    lhsT, rhs = w_sb[:, p, et, :], xt[:, et, :]
                                nc.tensor.matmul(psp[:], lhsT, rhs,
                                                 start=(et == 0), stop=(et == ET - 1))
                            for hp in range(2):
                                h = p * 2 + hp
                                dsl = bass.ts(hp, 64)
                                if which == "q":
                                    nc.scalar.activation(
                                        qh[h][0:64, sl], psp[dsl, :], AF.Identity,
                                        bias=bqs_sb[dsl, p:p + 1], scale=0.125)
                                    if SCORES_A_BF16:
                                        nc.scalar.activation(
                                            qa[h][:, sl], psp[dsl, :], AF.Identity,
                                            bias=bqs_sb[dsl, p:p + 1], scale=0.125)
                                elif which == "k":
                                    nc.scalar.activation(
                                        kh[h][0:64, sl], psp[dsl, :], AF.Identity,
                                        bias=bks_sb[dsl, p:p + 1], scale=1.0)
                                    if SCORES_A_BF16:
                                        nc.scalar.activation(
                                            ka[h][:, sl], psp[dsl, :], AF.Identity,
                                            bias=bks_sb[dsl, p:p + 1], scale=1.0)
                                else:
                                    nc.vector.scalar_tensor_tensor(
                                        out=vh[h][:, st, 0:64],
                                        in0=psp[:, dsl], scalar=1.0,
                                        in1=bvb_sb[:, p, dsl],
                                        op0=ALU.mult, op1=ALU.add)

            # ---- phase 2+3: attention + out-projection ----
            with (
                tc.tile_pool(name="small", bufs=8) as small,
                tc.tile_pool(name="attw", bufs=1) as att_pool,
                tc.tile_pool(name="ctxn", bufs=4) as ctx_pool,
                tc.tile_pool(name="ps_a", bufs=2, space="PSUM") as ps_a,
                tc.tile_pool(name="ps_b", bufs=2, space="PSUM") as ps_b,
                tc.tile_pool(name="ps_c", bufs=2, space="PSUM") as ps_c,
                tc.tile_pool(name="ps_t2", bufs=2, space="PSUM") as ps_t2,
            ):
                for ib in range(IB):
                    isl = bass.ts(ib, 512)
                    for h in range(HPC):
                        # A: approximate row max of logits
                        for it in range(4):
                            i0 = ib * 512 + it * 128
                            itsl = bass.ds(i0, 128)
                            nm2 = small.tile([128, 4], F32, name="nm2")
                            for jh in range(4):
                                psa = ps_a.tile([128, 512], F32, name="psa")
                                jsl = bass.ts(jh, 512)
                                if SCORES_A_BF16:
                                    nc.tensor.matmul(
                                        psa[:], qa[h][:, itsl], ka[h][:, jsl],
                                        start=True, stop=True)
                                else:
                                    nc.tensor.matmul(
                                        psa[:], qh[h][0:64, itsl], kh[h][0:64, jsl],
                                        start=True, stop=True)
                                nc.vector.reduce_max(nm2[:, jh:jh + 1], psa[:],
                                                     axis=AX.X)
                            nm = small.tile([128, 1], F32, name="nm")
                            nc.vector.reduce_max(nm[:], nm2[:], axis=AX.X,
                                                 negate=True)
                            pst2 = ps_t2.tile([64, 128], F32, name="pst2")
                            nc.tensor.transpose(pst2[0:1, :], nm[:], ident[:])
                            nc.scalar.activation(qh[h][64:65, itsl], pst2[0:1, :],
                                                 AF.Identity, bias=marg[0:1, 0:1],
                                                 scale=1.0)
                        # B: shifted transposed scores + exp
                        attT = att_pool.tile([128, JT, 512], ATT_DTYPE, name="attT")
                        for jt in range(JT):
                            psb = ps_b.tile([128, 512], F32, name="psb")
                            nc.tensor.matmul(psb[:], kh[h][:, bass.ts(jt, 128)],
                                             qh[h][:, isl], start=True, stop=True)
                            nc.scalar.activation(attT[:, jt, :], psb[:], AF.Exp)
                        # C: att @ V-hat, normalize, transpose ctx
                        for it in range(4):
                            i0 = ib * 512 + it * 128
                            psc = ps_c.tile([128, 65], F32, name="psc")
                            for jt in range(JT):
                                nc.tensor.matmul(psc[:], attT[:, jt, bass.ts(it, 128)],
                                                 vh[h][:, jt, :],
                                                 start=(jt == 0), stop=(jt == JT - 1))
                            recip = small.tile([128, 1], F32, name="recip")
                            nc.vector.reciprocal(recip[:], psc[:, 64:65])
                            ctxn = ctx_pool.tile([128, 64], F32, name="ctxn")
                            nc.vector.tensor_scalar_mul(ctxn[:], psc[:, 0:64],
                                                        recip[:])
                            pst2 = ps_t2.tile([64, 128], F32, name="pst2")
                            nc.tensor.transpose(pst2[:], ctxn[:], ident[:])
                            nc.vector.tensor_copy(
                                ctxT[h // 2][bass.ts(h % 2, 64), bass.ds(i0, 128)],
                                pst2[:])
                    # out-projection for this i-block
                    for it in range(4):
                        i0 = ib * 512 + it * 128
                        for eh in range(2):
                            pso = ps_b.tile([128, 512], F32, name="psb")
                            for ct in range(2):
                                nc.tensor.matmul(pso[:],
                                                 ctxT[ct][:, bass.ds(i0, 128)],
                                                 wo_sb[:, ct, bass.ts(eh, 512)],
                                                 start=(ct == 0), stop=(ct == 1))
                            outsb = ctx_pool.tile([128, 512], F32, name="outsb")
                            nc.scalar.copy(outsb[:], pso[:])
                            nc.sync.dma_start(out_p[bass.ds(i0, 128),
                                                    bass.ts(eh, 512)], outsb[:])
    nc.finalize()
    return nc


_NC_CACHE = None


def _get_nc():
    global _NC_CACHE
    if _NC_CACHE is None:
        _NC_CACHE = build_bass()
    return _NC_CACHE


def _prep_core_inputs(inputs, core):
    b, hg = core // 4, core % 4
    h0 = hg * HPC
    q, k, v = inputs["q"], inputs["k"], inputs["v"]
    Wq, Wk, Wv = inputs["Wq"], inputs["Wk"], inputs["Wv"]
    bq, bk, bv = inputs["bq"], inputs["bk"], inputs["bv"]
    Wo = inputs["Wo"]

    def pack_w(W):
        # [NPAIR, ET, 128, 128]: pair p, e-tile t -> [W[h0+2p] | W[h0+2p+1]]
        out = np.empty((NPAIR, ET, 128, 128), np.float32)
        for p in range(NPAIR):
            pair = np.concatenate([W[h0 + 2 * p], W[h0 + 2 * p + 1]], axis=1)
            out[p] = pair.reshape(ET, 128, 128)
        return out

    def pack_bcol(bias, scale):
        out = np.empty((128, NPAIR), np.float32)
        for p in range(NPAIR):
            out[:, p] = np.concatenate(
                [bias[h0 + 2 * p], bias[h0 + 2 * p + 1]]) * scale
        return out

    bvb = np.empty((128, NPAIR, 128), np.float32)
    for p in range(NPAIR):
        bvb[:, p, :] = np.concatenate([bv[h0 + 2 * p], bv[h0 + 2 * p + 1]])[None, :]

    wo_rows = Wo[h0 * DH:(h0 + HPC) * DH, :]  # [256, E]
    return {
        "x_q": np.ascontiguousarray(q[b]),
        "x_k": np.ascontiguousarray(k[b]),
        "x_v": np.ascontiguousarray(v[b]),
        "wq": pack_w(Wq), "wk": pack_w(Wk), "wv": pack_w(Wv),
        "bqs": pack_bcol(bq, 0.125), "bks": pack_bcol(bk, 1.0), "bvb": bvb,
        "wo": np.ascontiguousarray(wo_rows.reshape(NPAIR, 128, E)),
    }


def run(inputs, trace=False, **kw):
    inputs = {k: np.asarray(v) for k, v in inputs.items()}
    nc = _get_nc()
    in_maps = [_prep_core_inputs(inputs, c) for c in range(NCORES)]
    res = run_bass_kernel_spmd(nc, in_maps, list(range(NCORES)), trace=trace, **kw)
    bo = inputs["bo"]
    out = np.empty((B, S, E), np.float32)
    for b in range(B):
        acc = res.results[b * 4]["out_p"].astype(np.float32)
        for c in range(b * 4 + 1, b * 4 + 4):
            acc = acc + res.results[c]["out_p"]
        out[b] = acc + bo[None, :]
    return out, res


def kernel(**inputs):
    out, _ = run(inputs)
    return out


# revision 17
# speedup vs baseline: 1.5079x; 1.5079x over previous
"""Multi-head attention TRN2 Bass kernel.

Problem: B=2, S=2048, E=1024, H=16, Dh=64; per-head QKV projection weights,
unmasked softmax(Q K^T / sqrt(Dh)) @ V, concat heads, out-projection.

Sharding: 8 cores = 2 batches x 4 head-groups (4 heads each). Each core
computes its batch/head-group's attention and a partial out-projection
(the rows of Wo belonging to its heads); the host sums the 4 partials per
batch and adds bo.

Numerics: the attention logits here have std ~1200, so softmax is
near-one-hot and argmax flips dominate the error. The Q/K projection and
the scores matmul therefore run in fp32. The row-max used for the softmax
shift is computed from a bf16 scores pass (softmax is invariant to the
shift as long as it is within ~80 of the true max; we add a safety margin).
The max-subtraction is folded into the transposed-scores matmul via an
augmented contraction row, and the softmax denominator falls out of an
extra ones-column on V.
"""

import numpy as np

import concourse.bacc as bacc
import concourse.bass as bass
import concourse.mybir as mybir
import concourse.tile as tile
from concourse import masks
from concourse.bass_utils import run_bass_kernel_spmd

F32 = mybir.dt.float32
F32R = mybir.dt.float32r
BF16 = mybir.dt.bfloat16
AX = mybir.AxisListType
AF = mybir.ActivationFunctionType
ALU = mybir.AluOpType

B, S, E, H, DH = 2, 2048, 1024, 16, 64
NCORES = 8
HPC = 4          # heads per core
NPAIR = 2        # head pairs per core
ET = E // 128    # 8 e-tiles
ST = S // 128    # 16 s-tiles
IB = S // 512    # 4 i-blocks
JT = S // 128    # 16 j-tiles
MARGIN = 32.0    # safety margin for the approximate row max

# dtype of the V path (v transpose, V projection, att^T, att@V). BF16 runs
# those matmuls at 1 cycle/row vs fp32's 4; costs ~3e-3 absmax relative
# error. The Q/K path must stay fp32 (argmax flips).
ATT_DTYPE = BF16
SCORES_A_BF16 = True


def build_bass():
    nc = bacc.Bacc("TRN2", target_bir_lowering=False, debug=False,
                   num_devices=NCORES)
    vdt = ATT_DTYPE
    x_q = nc.dram_tensor("x_q", [S, E], F32, kind="ExternalInput")
    x_k = nc.dram_tensor("x_k", [S, E], F32, kind="ExternalInput")
    x_v = nc.dram_tensor("x_v", [S, E], vdt, kind="ExternalInput")
    wq = nc.dram_tensor("wq", [NPAIR, ET, 128, 128], F32, kind="ExternalInput")
    wk = nc.dram_tensor("wk", [NPAIR, ET, 128, 128], F32, kind="ExternalInput")
    wv = nc.dram_tensor("wv", [NPAIR, ET, 128, 128], vdt, kind="ExternalInput")
    bqs = nc.dram_tensor("bqs", [128, NPAIR], F32, kind="ExternalInput")
    bks = nc.dram_tensor("bks", [128, NPAIR], F32, kind="ExternalInput")
    bvb = nc.dram_tensor("bvb", [128, NPAIR, 128], F32, kind="ExternalInput")
    wo = nc.dram_tensor("wo", [NPAIR, 128, E], F32, kind="ExternalInput")
    out_p = nc.dram_tensor("out_p", [S, E], F32, kind="ExternalOutput")

    with tile.TileContext(nc) as tc:
        with (
            tc.tile_pool(name="const", bufs=1) as const_pool,
            tc.tile_pool(name="persist", bufs=1) as persist,
        ):
            ident = const_pool.tile([128, 128], F32, name="ident")
            masks.make_identity(nc, ident[:])
            ident_v = const_pool.tile([128, 128], vdt, name="ident_v")
            masks.make_identity(nc, ident_v[:])
            marg = const_pool.tile([128, 1], F32, name="marg")
            nc.gpsimd.memset(marg[:], -MARGIN)

            bqs_sb = const_pool.tile([128, NPAIR], F32, name="bqs")
            nc.sync.dma_start(bqs_sb[:], bqs[:])
            bks_sb = const_pool.tile([128, NPAIR], F32, name="bks")
            nc.sync.dma_start(bks_sb[:], bks[:])
            bvb_sb = const_pool.tile([128, NPAIR, 128], F32, name="bvb")
            nc.sync.dma_start(bvb_sb[:], bvb[:])
            # out-projection runs in float32r (~14-bit mantissa, ample for
            # this matmul); inputs must be explicitly rounded, so DMA to a
            # staging tile and round via a copy
            wo_st = const_pool.tile([128, NPAIR, E], F32, name="wo_st")
            nc.sync.dma_start(wo_st[:], wo.rearrange("c p e -> p c e"))
            wo_sb = const_pool.tile([128, NPAIR, E], F32R, name="wo")
            nc.vector.tensor_copy(wo_sb[:], wo_st[:])

            # persistent per-head tensors
            qh = [persist.tile([65, S], F32, name=f"qh{h}") for h in range(HPC)]
            kh = [persist.tile([65, S], F32, name=f"kh{h}") for h in range(HPC)]
            vh = [persist.tile([128, JT, 65], ATT_DTYPE, name=f"vh{h}")
                  for h in range(HPC)]
            if SCORES_A_BF16:
                qa = [persist.tile([64, S], BF16, name=f"qa{h}") for h in range(HPC)]
                ka = [persist.tile([64, S], BF16, name=f"ka{h}") for h in range(HPC)]
            ctxT = [persist.tile([128, S], F32R, name=f"ctxT{c}") for c in range(2)]

            for h in range(HPC):
                nc.gpsimd.memset(kh[h][64:65, :], 1.0)
                nc.gpsimd.memset(vh[h][:, :, 64:65], 1.0)

            # ---- phase 1: load, transpose, project ----
            with (
                tc.tile_pool(name="stage", bufs=3) as stage_pool,
                tc.tile_pool(name="xt", bufs=3) as xt_pool,
                tc.tile_pool(name="wght", bufs=1) as w_pool,
                tc.tile_pool(name="ps_tin", bufs=3, space="PSUM") as ps_tin,
                tc.tile_pool(name="ps_proj", bufs=4, space="PSUM") as ps_proj,
            ):
                for x_dram, w_dram, which in ((x_q, wq, "q"), (x_k, wk, "k"),
                                              (x_v, wv, "v")):
                    dt_in = vdt if which == "v" else F32
                    id_in = ident_v if which == "v" else ident
                    w_sb = w_pool.tile([128, NPAIR, ET, 128], dt_in, name="w_in")
                    nc.sync.dma_start(w_sb[:], w_dram.rearrange("p t e d -> e p t d"))
                    for st in range(ST):
                        sl = bass.ts(st, 128)
                        stage = stage_pool.tile([128, E], dt_in, name="stage")
                        nc.sync.dma_start(stage[:], x_dram[sl, :])
                        xt = xt_pool.tile([128, ET, 128], dt_in, name="xt")
                        for et in range(ET):
                            pst = ps_tin.tile([128, 128], dt_in, name="pst")
                            nc.tensor.transpose(pst[:], stage[:, bass.ts(et, 128)],
                                                id_in[:])
                            nc.vector.tensor_copy(xt[:, et, :], pst[:])
                        for p in range(NPAIR):
                            psp = ps_proj.tile([128, 128], F32, name="psp")
                            for et in range(ET):
                                if which == "v":
                                    lhsT, rhs = xt[:, et, :], w_sb[:, p, et, :]
                                else:
                                    lhsT, rhs = w_sb[:, p, et, :], xt[:, et, :]
                                nc.tensor.matmul(psp[:], lhsT, rhs,
                                                 start=(et == 0), stop=(et == ET - 1))
                            for hp in range(2):
                                h = p * 2 + hp
                                dsl = bass.ts(hp, 64)
                                if which == "q":
                                    nc.scalar.activation(
                                        qh[h][0:64, sl], psp[dsl, :], AF.Identity,
                                        bias=bqs_sb[dsl, p:p + 1], scale=0.125)
                                    if SCORES_A_BF16:
                                        nc.scalar.activation(
                                            qa[h][:, sl], psp[dsl, :], AF.Identity,
                                            bias=bqs_sb[dsl, p:p + 1], scale=0.125)
                                elif which == "k":
                                    nc.scalar.activation(
                                        kh[h][0:64, sl], psp[dsl, :], AF.Identity,
                                        bias=bks_sb[dsl, p:p + 1], scale=1.0)
                                    if SCORES_A_BF16:
                                        nc.scalar.activation(
                                            ka[h][:, sl], psp[dsl, :], AF.Identity,
                                            bias=bks_sb[dsl, p:p + 1], scale=1.0)
                                else:
                                    nc.vector.scalar_tensor_tensor(
                                        out=vh[h][:, st, 0:64],
                                        in0=psp[:, dsl], scalar=1.0,
                                        in1=bvb_sb[:, p, dsl],
                                        op0=ALU.mult, op1=ALU.add)

            # ---- phase 2+3: attention + out-projection ----
            with (
                tc.tile_pool(name="small", bufs=8) as small,
                tc.tile_pool(name="attw", bufs=1) as att_pool,
                tc.tile_pool(name="ctxn", bufs=4) as ctx_pool,
                tc.tile_pool(name="ps_a", bufs=2, space="PSUM") as ps_a,
                tc.tile_pool(name="ps_b", bufs=2, space="PSUM") as ps_b,
                tc.tile_pool(name="ps_c", bufs=2, space="PSUM") as ps_c,
                tc.tile_pool(name="ps_t2", bufs=2, space="PSUM") as ps_t2,
            ):
                for ib in range(IB):
                    isl = bass.ts(ib, 512)
                    for h in range(HPC):
                        # A: approximate row max of logits
                        for it in range(4):
                            i0 = ib * 512 + it * 128
                            itsl = bass.ds(i0, 128)
                            nm2 = small.tile([128, 4], F32, name="nm2")
                            for jh in range(4):
                                psa = ps_a.tile([128, 512], F32, name="psa")
                                jsl = bass.ts(jh, 512)
                                if SCORES_A_BF16:
                                    nc.tensor.matmul(
                                        psa[:], qa[h][:, itsl], ka[h][:, jsl],
                                        start=True, stop=True)
                                else:
                                    nc.tensor.matmul(
                                        psa[:], qh[h][0:64, itsl], kh[h][0:64, jsl],
                                        start=True, stop=True)
                                nc.vector.reduce_max(nm2[:, jh:jh + 1], psa[:],
                                                     axis=AX.X)
                            nm = small.tile([128, 1], F32, name="nm")
                            nc.vector.reduce_max(nm[:], nm2[:], axis=AX.X,
                                                 negate=True)
                            pst2 = ps_t2.tile([64, 128], F32, name="pst2")
                            nc.tensor.transpose(pst2[0:1, :], nm[:], ident[:])
                            nc.scalar.activation(qh[h][64:65, itsl], pst2[0:1, :],
                                                 AF.Identity, bias=marg[0:1, 0:1],
                                                 scale=1.0)
                        # B: shifted transposed scores + exp
                        attT = att_pool.tile([128, JT, 512], ATT_DTYPE, name="attT")
                        for jt in range(JT):
                            psb = ps_b.tile([128, 512], F32, name="psb")
                            nc.tensor.matmul(psb[:], kh[h][:, bass.ts(jt, 128)],
                                             qh[h][:, isl], start=True, stop=True)
                            nc.scalar.activation(attT[:, jt, :], psb[:], AF.Exp)
                        # C: att @ V-hat, normalize, transpose ctx
                        for it in range(4):
                            i0 = ib * 512 + it * 128
                            psc = ps_c.tile([128, 65], F32, name="psc")
                            for jt in range(JT):
                                nc.tensor.matmul(psc[:], attT[:, jt, bass.ts(it, 128)],
                                                 vh[h][:, jt, :],
                                                 start=(jt == 0), stop=(jt == JT - 1))
                            recip = small.tile([128, 1], F32, name="recip")
                            nc.vector.reciprocal(recip[:], psc[:, 64:65])
                            ctxn = ctx_pool.tile([128, 64], F32, name="ctxn")
                            nc.vector.tensor_scalar_mul(ctxn[:], psc[:, 0:64],
                                                        recip[:])
                            pst2 = ps_t2.tile([64, 128], F32, name="pst2")
                            nc.tensor.transpose(pst2[:], ctxn[:], ident[:])
                            nc.vector.tensor_copy(
                                ctxT[h // 2][bass.ts(h % 2, 64), bass.ds(i0, 128)],
                                pst2[:])
                    # out-projection for this i-block
                    for it in range(4):
                        i0 = ib * 512 + it * 128
                        for eh in range(2):
                            pso = ps_b.tile([128, 512], F32, name="psb")
                            for ct in range(2):
                                nc.tensor.matmul(pso[:],
                                                 ctxT[ct][:, bass.ds(i0, 128)],
                                                 wo_sb[:, ct, bass.ts(eh, 512)],
                                                 start=(ct == 0), stop=(ct == 1))
                            outsb = ctx_pool.tile([128, 512], F32, name="outsb")
                            nc.scalar.copy(outsb[:], pso[:])
                            nc.sync.dma_start(out_p[bass.ds(i0, 128),
                                                    bass.ts(eh, 512)], outsb[:])
    nc.finalize()
    return nc


_NC_CACHE = None


def _get_nc():
    global _NC_CACHE
    if _NC_CACHE is None:
        _NC_CACHE = build_bass()
    return _NC_CACHE


def _prep_core_inputs(inputs, core):
    b, hg = core // 4, core % 4
    h0 = hg * HPC
    q, k, v = inputs["q"], inputs["k"], inputs["v"]
    Wq, Wk, Wv = inputs["Wq"], inputs["Wk"], inputs["Wv"]
    bq, bk, bv = inputs["bq"], inputs["bk"], inputs["bv"]
    Wo = inputs["Wo"]

    def pack_w(W):
        # [NPAIR, ET, 128, 128]: pair p, e-tile t -> [W[h0+2p] | W[h0+2p+1]]
        out = np.empty((NPAIR, ET, 128, 128), np.float32)
        for p in range(NPAIR):
            pair = np.concatenate([W[h0 + 2 * p], W[h0 + 2 * p + 1]], axis=1)
            out[p] = pair.reshape(ET, 128, 128)
        return out

    def pack_bcol(bias, scale):
        out = np.empty((128, NPAIR), np.float32)
        for p in range(NPAIR):
            out[:, p] = np.concatenate(
                [bias[h0 + 2 * p], bias[h0 + 2 * p + 1]]) * scale
        return out

    bvb = np.empty((128, NPAIR, 128), np.float32)
    for p in range(NPAIR):
        bvb[:, p, :] = np.concatenate([bv[h0 + 2 * p], bv[h0 + 2 * p + 1]])[None, :]

    wo_rows = Wo[h0 * DH:(h0 + HPC) * DH, :]  # [256, E]
    vdt_np = mybir.dt.np(ATT_DTYPE)
    return {
        "x_q": np.ascontiguousarray(q[b]),
        "x_k": np.ascontiguousarray(k[b]),
        "x_v": np.ascontiguousarray(v[b]).astype(vdt_np),
        "wq": pack_w(Wq), "wk": pack_w(Wk),
        "wv": pack_w(Wv).astype(vdt_np),
        "bqs": pack_bcol(bq, 0.125), "bks": pack_bcol(bk, 1.0), "bvb": bvb,
        "wo": np.ascontiguousarray(wo_rows.reshape(NPAIR, 128, E)),
    }


def run(inputs, trace=False, **kw):
    inputs = {k: np.asarray(v) for k, v in inputs.items()}
    nc = _get_nc()
    in_maps = [_prep_core_inputs(inputs, c) for c in range(NCORES)]
    res = run_bass_kernel_spmd(nc, in_maps, list(range(NCORES)), trace=trace, **kw)
    bo = inputs["bo"]
    out = np.empty((B, S, E), np.float32)
    for b in range(B):
        acc = res.results[b * 4]["out_p"].astype(np.float32)
        for c in range(b * 4 + 1, b * 4 + 4):
            acc = acc + res.results[c]["out_p"]
        out[b] = acc + bo[None, :]
    return out, res


def kernel(**inputs):
    out, _ = run(inputs)
    return out


# revision 25
# speedup vs baseline: 1.5814x; 1.0488x over previous
"""Multi-head attention TRN2 Bass kernel.

Problem: B=2, S=2048, E=1024, H=16, Dh=64; per-head QKV projection weights,
unmasked softmax(Q K^T / sqrt(Dh)) @ V, concat heads, out-projection.

Sharding: 8 cores = 2 batches x 4 head-groups (4 heads each). Each core
computes its batch/head-group's attention and a partial out-projection
(the rows of Wo belonging to its heads); the host sums the 4 partials per
batch and adds bo.

Numerics: the attention logits here have std ~1200, so softmax is
near-one-hot and argmax flips dominate the error; the Q/K/scores path
needs fp32-level precision. Scores run as a 3-term float32r hi/lo
decomposition (Qr·Kr + Qr·Kl + Ql·Kr) which is fp32-accurate but runs at
1 cycle/row instead of fp32's 4. Heads are packed in pairs on partitions
0:64 / 64:128 so the two heads' 64-row score matmuls run concurrently in
the PE array (row groups). The softmax row-max (from a cheap hi-only
scores pass) is subtracted via an accumulated rank-1 matmul, and the
softmax denominator falls out of an extra ones-column on V. The V path
(v transpose, V projection, att^T, att@V) runs in bf16; the
out-projection in float32r.
"""

import numpy as np

import concourse.bacc as bacc
import concourse.bass as bass
import concourse.mybir as mybir
import concourse.tile as tile
from concourse import masks
from concourse.bass_utils import run_bass_kernel_spmd

F32 = mybir.dt.float32
F32R = mybir.dt.float32r
BF16 = mybir.dt.bfloat16
AX = mybir.AxisListType
AF = mybir.ActivationFunctionType
ALU = mybir.AluOpType

B, S, E, H, DH = 2, 2048, 1024, 16, 64
NCORES = 8
HPC = 4          # heads per core
NPAIR = 2        # head pairs per core
ET = E // 128    # 8 e-tiles
ST = S // 128    # 16 s-tiles
IB = S // 512    # 4 i-blocks
JT = S // 128    # 16 j-tiles
MARGIN = 32.0    # safety margin for the approximate row max

ATT_DTYPE = BF16  # V path dtype


def build_bass():
    nc = bacc.Bacc("TRN2", target_bir_lowering=False, debug=False,
                   num_devices=NCORES)
    vdt = ATT_DTYPE
    x_q = nc.dram_tensor("x_q", [S, E], F32, kind="ExternalInput")
    x_k = nc.dram_tensor("x_k", [S, E], F32, kind="ExternalInput")
    x_v = nc.dram_tensor("x_v", [S, E], vdt, kind="ExternalInput")
    wq = nc.dram_tensor("wq", [NPAIR, ET, 128, 128], F32, kind="ExternalInput")
    wk = nc.dram_tensor("wk", [NPAIR, ET, 128, 128], F32, kind="ExternalInput")
    wv = nc.dram_tensor("wv", [NPAIR, ET, 128, 128], vdt, kind="ExternalInput")
    bqs = nc.dram_tensor("bqs", [128, NPAIR], F32, kind="ExternalInput")
    bks = nc.dram_tensor("bks", [128, NPAIR], F32, kind="ExternalInput")
    bvb = nc.dram_tensor("bvb", [128, NPAIR, 128], F32, kind="ExternalInput")
    wo = nc.dram_tensor("wo", [NPAIR, 128, E], F32, kind="ExternalInput")
    out_p = nc.dram_tensor("out_p", [S, E], F32, kind="ExternalOutput")

    with tile.TileContext(nc) as tc:
        with (
            tc.tile_pool(name="const", bufs=1) as const_pool,
            tc.tile_pool(name="persist", bufs=1) as persist,
        ):
            ident = const_pool.tile([128, 128], F32, name="ident")
            masks.make_identity(nc, ident[:])
            ident_v = const_pool.tile([128, 128], vdt, name="ident_v")
            masks.make_identity(nc, ident_v[:])
            marg = const_pool.tile([128, 1], F32, name="marg")
            nc.gpsimd.memset(marg[:], -MARGIN)
            # all-ones f32r tile for the rank-1 (-m) accumulate; rounded via
            # a copy so the f32r matmult verifier accepts it
            ones_st = const_pool.tile([128, 128], F32, name="ones_st")
            nc.gpsimd.memset(ones_st[:], 1.0)
            ones_r = const_pool.tile([128, 128], F32R, name="ones_r")
            nc.vector.tensor_copy(ones_r[:], ones_st[:])

            bqs_sb = const_pool.tile([128, NPAIR], F32, name="bqs")
            nc.sync.dma_start(bqs_sb[:], bqs[:])
            bks_sb = const_pool.tile([128, NPAIR], F32, name="bks")
            nc.sync.dma_start(bks_sb[:], bks[:])
            bvb_sb = const_pool.tile([128, NPAIR, 128], F32, name="bvb")
            nc.sync.dma_start(bvb_sb[:], bvb[:])
            # out-projection runs in float32r; round via copy
            wo_st = const_pool.tile([128, NPAIR, E], F32, name="wo_st")
            nc.sync.dma_start(wo_st[:], wo.rearrange("c p e -> p c e"))
            wo_sb = const_pool.tile([128, NPAIR, E], F32R, name="wo")
            nc.vector.tensor_copy(wo_sb[:], wo_st[:])

            # persistent per-pair packed tensors (rows 0:64 = even head,
            # rows 64:128 = odd head of the pair)
            qr = [persist.tile([128, S], F32R, name=f"qr{p}") for p in range(NPAIR)]
            ql = [persist.tile([128, S], F32R, name=f"ql{p}") for p in range(NPAIR)]
            kr = [persist.tile([128, S], F32R, name=f"kr{p}") for p in range(NPAIR)]
            kl = [persist.tile([128, S], F32R, name=f"kl{p}") for p in range(NPAIR)]
            # -(rowmax)-MARGIN per head, single-partition tiles (rank-1 rhs
            # and PE-transpose outputs must sit at partition 0)
            nm_h = [persist.tile([1, S], F32R, name=f"nm{h}") for h in range(HPC)]
            vh = [persist.tile([128, JT, 65], vdt, name=f"vh{h}")
                  for h in range(HPC)]
            ctxT = [persist.tile([128, S], F32R, name=f"ctxT{c}") for c in range(2)]

            for h in range(HPC):
                nc.gpsimd.memset(vh[h][:, :, 64:65], 1.0)

            # ---- phase 1: load, transpose, project ----
            with (
                tc.tile_pool(name="stage", bufs=3) as stage_pool,
                tc.tile_pool(name="xt", bufs=3) as xt_pool,
                tc.tile_pool(name="wght", bufs=1) as w_pool,
                tc.tile_pool(name="scr", bufs=3) as scr_pool,
                tc.tile_pool(name="ps_tin", bufs=3, space="PSUM") as ps_tin,
                tc.tile_pool(name="ps_proj", bufs=4, space="PSUM") as ps_proj,
            ):
                for x_dram, w_dram, which in ((x_q, wq, "q"), (x_k, wk, "k"),
                                              (x_v, wv, "v")):
                    dt_in = vdt if which == "v" else F32
                    id_in = ident_v if which == "v" else ident
                    w_sb = w_pool.tile([128, NPAIR, ET, 128], dt_in, name="w_in")
                    nc.sync.dma_start(w_sb[:], w_dram.rearrange("p t e d -> e p t d"))
                    for st in range(ST):
                        sl = bass.ts(st, 128)
                        stage = stage_pool.tile([128, E], dt_in, name="stage")
                        nc.sync.dma_start(stage[:], x_dram[sl, :])
                        xt = xt_pool.tile([128, ET, 128], dt_in, name="xt")
                        for et in range(ET):
                            pst = ps_tin.tile([128, 128], dt_in, name="pst")
                            nc.tensor.transpose(pst[:], stage[:, bass.ts(et, 128)],
                                                id_in[:])
                            nc.vector.tensor_copy(xt[:, et, :], pst[:])
                        for p in range(NPAIR):
                            psp = ps_proj.tile([128, 128], F32, name="psp")
                            for et in range(ET):
                                if which == "v":
                                    lhsT, rhs = xt[:, et, :], w_sb[:, p, et, :]
                                else:
                                    lhsT, rhs = w_sb[:, p, et, :], xt[:, et, :]
                                nc.tensor.matmul(psp[:], lhsT, rhs,
                                                 start=(et == 0), stop=(et == ET - 1))
                            if which == "q":
                                qex = scr_pool.tile([128, 128], F32, name="qex")
                                nc.scalar.activation(qex[:], psp[:], AF.Identity,
                                                     bias=bqs_sb[:, p:p + 1],
                                                     scale=0.125)
                                nc.vector.tensor_copy(qr[p][:, sl], qex[:])
                                nc.vector.scalar_tensor_tensor(
                                    out=ql[p][:, sl], in0=qex[:], scalar=1.0,
                                    in1=qr[p][:, sl],
                                    op0=ALU.mult, op1=ALU.subtract)
                            elif which == "k":
                                kex = scr_pool.tile([128, 128], F32, name="qex")
                                nc.scalar.activation(kex[:], psp[:], AF.Identity,
                                                     bias=bks_sb[:, p:p + 1],
                                                     scale=1.0)
                                nc.vector.tensor_copy(kr[p][:, sl], kex[:])
                                nc.vector.scalar_tensor_tensor(
                                    out=kl[p][:, sl], in0=kex[:], scalar=1.0,
                                    in1=kr[p][:, sl],
                                    op0=ALU.mult, op1=ALU.subtract)
                            else:
                                for hp in range(2):
                                    h = p * 2 + hp
                                    dsl = bass.ts(hp, 64)
                                    nc.vector.scalar_tensor_tensor(
                                        out=vh[h][:, st, 0:64],
                                        in0=psp[:, dsl], scalar=1.0,
                                        in1=bvb_sb[:, p, dsl],
                                        op0=ALU.mult, op1=ALU.add)

            # ---- phase 2+3: attention + out-projection ----
            with (
                tc.tile_pool(name="small", bufs=8) as small,
                tc.tile_pool(name="attw", bufs=1) as att_pool,
                tc.tile_pool(name="ctxn", bufs=4) as ctx_pool,
                tc.tile_pool(name="ps_a", bufs=2, space="PSUM") as ps_a,
                tc.tile_pool(name="ps_b", bufs=4, space="PSUM") as ps_b,
                tc.tile_pool(name="ps_m", bufs=2, space="PSUM") as ps_m,
            ):
                for ib in range(IB):
                    isl = bass.ts(ib, 512)
                    for p in range(NPAIR):
                        rows = [bass.ts(0, 64), bass.ts(1, 64)]
                        # A: approximate row max of logits (hi parts only);
                        # the two heads' 64-row matmuls pair in the array
                        for it in range(4):
                            i0 = ib * 512 + it * 128
                            itsl = bass.ds(i0, 128)
                            nm2 = [small.tile([128, 4], F32, name="nm2")
                                   for hp in range(2)]
                            for jh in range(4):
                                jsl = bass.ts(jh, 512)
                                psa = [ps_a.tile([128, 512], F32, name="psa")
                                       for hp in range(2)]
                                for hp in range(2):
                                    nc.tensor.matmul(
                                        psa[hp][:], qr[p][rows[hp], itsl],
                                        kr[p][rows[hp], jsl],
                                        start=True, stop=True)
                                for hp in range(2):
                                    nc.vector.reduce_max(
                                        nm2[hp][:, jh:jh + 1], psa[hp][:],
                                        axis=AX.X)
                            for hp in range(2):
                                nm = small.tile([128, 1], F32, name="nm")
                                nc.vector.reduce_max(nm[:], nm2[hp][:], axis=AX.X,
                                                     negate=True)
                                pst2 = ps_m.tile([128, 128], F32, name="pst2")
                                nc.tensor.transpose(pst2[0:1, :], nm[:],
                                                    ident[:])
                                nc.scalar.activation(
                                    nm_h[p * 2 + hp][0:1, itsl], pst2[0:1, :],
                                    AF.Identity, bias=marg[0:1, 0:1], scale=1.0)
                        # B: shifted scores, 3-term f32r hi/lo, head-paired;
                        # -m applied via paired rank-1 accumulate; then exp
                        attT = [att_pool.tile([128, JT, 512], vdt, name=f"attT{hp}")
                                for hp in range(2)]
                        for jt in range(JT):
                            jsl = bass.ts(jt, 128)
                            psb = [ps_b.tile([128, 512], F32, name="psb")
                                   for hp in range(2)]
                            for hp in range(2):
                                nc.tensor.matmul(psb[hp][:], kr[p][rows[hp], jsl],
                                                 qr[p][rows[hp], isl],
                                                 start=True, stop=False)
                            for hp in range(2):
                                nc.tensor.matmul(psb[hp][:], kr[p][rows[hp], jsl],
                                                 ql[p][rows[hp], isl],
                                                 start=False, stop=False)
                            for hp in range(2):
                                nc.tensor.matmul(psb[hp][:], kl[p][rows[hp], jsl],
                                                 qr[p][rows[hp], isl],
                                                 start=False, stop=False)
                            for hp in range(2):
                                nc.tensor.matmul(psb[hp][:],
                                                 ones_r[0:1, 0:128],
                                                 nm_h[p * 2 + hp][0:1, isl],
                                                 start=False, stop=True)
                            for hp in range(2):
                                nc.scalar.activation(attT[hp][:, jt, :],
                                                     psb[hp][:], AF.Exp)
                        # C: att @ V-hat, normalize, transpose ctx
                        for hp in range(2):
                            h = p * 2 + hp
                            for it in range(4):
                                i0 = ib * 512 + it * 128
                                psc = ps_m.tile([128, 128], F32, name="pst2")
                                for jt in range(JT):
                                    nc.tensor.matmul(
                                        psc[:, 0:65],
                                        attT[hp][:, jt, bass.ts(it, 128)],
                                        vh[h][:, jt, :],
                                        start=(jt == 0), stop=(jt == JT - 1))
                                recip = small.tile([128, 1], F32, name="recip")
                                nc.vector.reciprocal(recip[:], psc[:, 64:65])
                                ctxn = ctx_pool.tile([128, 64], F32, name="ctxn")
                                nc.vector.tensor_scalar_mul(ctxn[:], psc[:, 0:64],
                                                            recip[:])
                                pst2 = ps_m.tile([128, 128], F32, name="pst2")
                                nc.tensor.transpose(pst2[0:64, :], ctxn[:],
                                                    ident[:])
                                nc.vector.tensor_copy(
                                    ctxT[h // 2][bass.ts(h % 2, 64),
                                                 bass.ds(i0, 128)],
                                    pst2[0:64, :])
                    # out-projection for this i-block (float32r)
                    for it in range(4):
                        i0 = ib * 512 + it * 128
                        for eh in range(2):
                            pso = ps_b.tile([128, 512], F32, name="psb")
                            for ct in range(2):
                                nc.tensor.matmul(pso[:],
                                                 ctxT[ct][:, bass.ds(i0, 128)],
                                                 wo_sb[:, ct, bass.ts(eh, 512)],
                                                 start=(ct == 0), stop=(ct == 1))
                            outsb = ctx_pool.tile([128, 512], F32, name="outsb")
                            nc.scalar.copy(outsb[:], pso[:])
                            nc.sync.dma_start(out_p[bass.ds(i0, 128),
                                                    bass.ts(eh, 512)], outsb[:])
    nc.finalize()
    return nc


_NC_CACHE = None


def _get_nc():
    global _NC_CACHE
    if _NC_CACHE is None:
        _NC_CACHE = build_bass()
    return _NC_CACHE


def _prep_core_inputs(inputs, core):
    b, hg = core // 4, core % 4
    h0 = hg * HPC
    q, k, v = inputs["q"], inputs["k"], inputs["v"]
    Wq, Wk, Wv = inputs["Wq"], inputs["Wk"], inputs["Wv"]
    bq, bk, bv = inputs["bq"], inputs["bk"], inputs["bv"]
    Wo = inputs["Wo"]

    def pack_w(W):
        # [NPAIR, ET, 128, 128]: pair p, e-tile t -> [W[h0+2p] | W[h0+2p+1]]
        out = np.empty((NPAIR, ET, 128, 128), np.float32)
        for p in range(NPAIR):
            pair = np.concatenate([W[h0 + 2 * p], W[h0 + 2 * p + 1]], axis=1)
            out[p] = pair.reshape(ET, 128, 128)
        return out

    def pack_bcol(bias, scale):
        out = np.empty((128, NPAIR), np.float32)
        for p in range(NPAIR):
            out[:, p] = np.concatenate(
                [bias[h0 + 2 * p], bias[h0 + 2 * p + 1]]) * scale
        return out

    bvb = np.empty((128, NPAIR, 128), np.float32)
    for p in range(NPAIR):
        bvb[:, p, :] = np.concatenate([bv[h0 + 2 * p], bv[h0 + 2 * p + 1]])[None, :]

    wo_rows = Wo[h0 * DH:(h0 + HPC) * DH, :]  # [256, E]
    vdt_np = mybir.dt.np(ATT_DTYPE)
    return {
        "x_q": np.ascontiguousarray(q[b]),
        "x_k": np.ascontiguousarray(k[b]),
        "x_v": np.ascontiguousarray(v[b]).astype(vdt_np),
        "wq": pack_w(Wq), "wk": pack_w(Wk),
        "wv": pack_w(Wv).astype(vdt_np),
        "bqs": pack_bcol(bq, 0.125), "bks": pack_bcol(bk, 1.0), "bvb": bvb,
        "wo": np.ascontiguousarray(wo_rows.reshape(NPAIR, 128, E)),
    }


def run(inputs, trace=False, **kw):
    inputs = {k: np.asarray(v) for k, v in inputs.items()}
    nc = _get_nc()
    in_maps = [_prep_core_inputs(inputs, c) for c in range(NCORES)]
    res = run_bass_kernel_spmd(nc, in_maps, list(range(NCORES)), trace=trace, **kw)
    bo = inputs["bo"]
    out = np.empty((B, S, E), np.float32)
    for b in range(B):
        acc = res.results[b * 4]["out_p"].astype(np.float32)
        for c in range(b * 4 + 1, b * 4 + 4):
            acc = acc + res.results[c]["out_p"]
        out[b] = acc + bo[None, :]
    return out, res


def kernel(**inputs):
    out, _ = run(inputs)
    return out


# revision 26
# speedup vs baseline: 1.8002x; 1.1383x over previous
"""Multi-head attention TRN2 Bass kernel.

Problem: B=2, S=2048, E=1024, H=16, Dh=64; per-head QKV projection weights,
unmasked softmax(Q K^T / sqrt(Dh)) @ V, concat heads, out-projection.

Sharding: 8 cores = 2 batches x 4 head-groups (4 heads each). Each core
computes its batch/head-group's attention and a partial out-projection
(the rows of Wo belonging to its heads); the host sums the 4 partials per
batch and adds bo.

Numerics: the attention logits here have std ~1200, so softmax is
near-one-hot and argmax flips dominate the error; the Q/K/scores path
needs fp32-level precision. Scores run as a 3-term float32r hi/lo
decomposition (Qr·Kr + Qr·Kl + Ql·Kr) which is fp32-accurate but runs at
1 cycle/row instead of fp32's 4. Heads are packed in pairs on partitions
0:64 / 64:128 so the two heads' 64-row score matmuls run concurrently in
the PE array (row groups). The softmax row-max (from a cheap hi-only
scores pass) is subtracted via an accumulated rank-1 matmul, and the
softmax denominator falls out of an extra ones-column on V. The V path
(v transpose, V projection, att^T, att@V) runs in bf16; the
out-projection in float32r.
"""

import numpy as np

import concourse.bacc as bacc
import concourse.bass as bass
import concourse.mybir as mybir
import concourse.tile as tile
from concourse import masks
from concourse.bass_utils import run_bass_kernel_spmd

F32 = mybir.dt.float32
F32R = mybir.dt.float32r
BF16 = mybir.dt.bfloat16
AX = mybir.AxisListType
AF = mybir.ActivationFunctionType
ALU = mybir.AluOpType

B, S, E, H, DH = 2, 2048, 1024, 16, 64
NCORES = 8
HPC = 4          # heads per core
NPAIR = 2        # head pairs per core
ET = E // 128    # 8 e-tiles
ST = S // 128    # 16 s-tiles
IB = S // 512    # 4 i-blocks
JT = S // 128    # 16 j-tiles
MARGIN = 32.0    # safety margin for the approximate row max

ATT_DTYPE = BF16  # V path dtype


def build_bass():
    nc = bacc.Bacc("TRN2", target_bir_lowering=False, debug=False,
                   num_devices=NCORES)
    vdt = ATT_DTYPE
    x_q = nc.dram_tensor("x_q", [S, E], F32, kind="ExternalInput")
    x_k = nc.dram_tensor("x_k", [S, E], F32, kind="ExternalInput")
    x_v = nc.dram_tensor("x_v", [S, E], vdt, kind="ExternalInput")
    wq = nc.dram_tensor("wq", [NPAIR, ET, 128, 128], F32, kind="ExternalInput")
    wk = nc.dram_tensor("wk", [NPAIR, ET, 128, 128], F32, kind="ExternalInput")
    wv = nc.dram_tensor("wv", [NPAIR, ET, 128, 128], vdt, kind="ExternalInput")
    bqs = nc.dram_tensor("bqs", [128, NPAIR], F32, kind="ExternalInput")
    bks = nc.dram_tensor("bks", [128, NPAIR], F32, kind="ExternalInput")
    bvb = nc.dram_tensor("bvb", [128, NPAIR, 128], F32, kind="ExternalInput")
    wo = nc.dram_tensor("wo", [NPAIR, 128, E], F32, kind="ExternalInput")
    out_p = nc.dram_tensor("out_p", [S, E], F32, kind="ExternalOutput")

    with tile.TileContext(nc) as tc:
        with (
            tc.tile_pool(name="const", bufs=1) as const_pool,
            tc.tile_pool(name="persist", bufs=1) as persist,
        ):
            ident = const_pool.tile([128, 128], F32, name="ident")
            masks.make_identity(nc, ident[:])
            ident_v = const_pool.tile([128, 128], vdt, name="ident_v")
            masks.make_identity(nc, ident_v[:])
            marg = const_pool.tile([128, 1], F32, name="marg")
            nc.gpsimd.memset(marg[:], -MARGIN)
            # all-ones f32r tile for the rank-1 (-m) accumulate; rounded via
            # a copy so the f32r matmult verifier accepts it
            ones_st = const_pool.tile([128, 128], F32, name="ones_st")
            nc.gpsimd.memset(ones_st[:], 1.0)
            ones_r = const_pool.tile([128, 128], BF16, name="ones_r")
            nc.vector.tensor_copy(ones_r[:], ones_st[:])

            bqs_sb = const_pool.tile([128, NPAIR], F32, name="bqs")
            nc.sync.dma_start(bqs_sb[:], bqs[:])
            bks_sb = const_pool.tile([128, NPAIR], F32, name="bks")
            nc.sync.dma_start(bks_sb[:], bks[:])
            bvb_sb = const_pool.tile([128, NPAIR, 128], F32, name="bvb")
            nc.sync.dma_start(bvb_sb[:], bvb[:])
            # out-projection runs in float32r; round via copy
            wo_st = const_pool.tile([128, NPAIR, E], F32, name="wo_st")
            nc.sync.dma_start(wo_st[:], wo.rearrange("c p e -> p c e"))
            wo_sb = const_pool.tile([128, NPAIR, E], F32R, name="wo")
            nc.vector.tensor_copy(wo_sb[:], wo_st[:])

            # persistent per-pair packed tensors (rows 0:64 = even head,
            # rows 64:128 = odd head of the pair)
            qr = [persist.tile([128, S], BF16, name=f"qr{p}") for p in range(NPAIR)]
            ql = [persist.tile([128, S], BF16, name=f"ql{p}") for p in range(NPAIR)]
            kr = [persist.tile([128, S], BF16, name=f"kr{p}") for p in range(NPAIR)]
            kl = [persist.tile([128, S], BF16, name=f"kl{p}") for p in range(NPAIR)]
            # -(rowmax)-MARGIN per head, single-partition tiles (rank-1 rhs
            # and PE-transpose outputs must sit at partition 0)
            nm_h = [persist.tile([1, S], BF16, name=f"nm{h}") for h in range(HPC)]
            vh = [persist.tile([128, JT, 65], vdt, name=f"vh{h}")
                  for h in range(HPC)]
            ctxT = [persist.tile([128, S], F32R, name=f"ctxT{c}") for c in range(2)]

            for h in range(HPC):
                nc.gpsimd.memset(vh[h][:, :, 64:65], 1.0)

            # ---- phase 1: load, transpose, project ----
            with (
                tc.tile_pool(name="stage", bufs=3) as stage_pool,
                tc.tile_pool(name="xt", bufs=3) as xt_pool,
                tc.tile_pool(name="wght", bufs=1) as w_pool,
                tc.tile_pool(name="scr", bufs=3) as scr_pool,
                tc.tile_pool(name="ps_tin", bufs=3, space="PSUM") as ps_tin,
                tc.tile_pool(name="ps_proj", bufs=4, space="PSUM") as ps_proj,
            ):
                for x_dram, w_dram, which in ((x_q, wq, "q"), (x_k, wk, "k"),
                                              (x_v, wv, "v")):
                    dt_in = vdt if which == "v" else F32
                    id_in = ident_v if which == "v" else ident
                    w_sb = w_pool.tile([128, NPAIR, ET, 128], dt_in, name="w_in")
                    nc.sync.dma_start(w_sb[:], w_dram.rearrange("p t e d -> e p t d"))
                    for st in range(ST):
                        sl = bass.ts(st, 128)
                        stage = stage_pool.tile([128, E], dt_in, name="stage")
                        nc.sync.dma_start(stage[:], x_dram[sl, :])
                        xt = xt_pool.tile([128, ET, 128], dt_in, name="xt")
                        for et in range(ET):
                            pst = ps_tin.tile([128, 128], dt_in, name="pst")
                            nc.tensor.transpose(pst[:], stage[:, bass.ts(et, 128)],
                                                id_in[:])
                            nc.vector.tensor_copy(xt[:, et, :], pst[:])
                        for p in range(NPAIR):
                            psp = ps_proj.tile([128, 128], F32, name="psp")
                            for et in range(ET):
                                if which == "v":
                                    lhsT, rhs = xt[:, et, :], w_sb[:, p, et, :]
                                else:
                                    lhsT, rhs = w_sb[:, p, et, :], xt[:, et, :]
                                nc.tensor.matmul(psp[:], lhsT, rhs,
                                                 start=(et == 0), stop=(et == ET - 1))
                            if which == "q":
                                qex = scr_pool.tile([128, 128], F32, name="qex")
                                nc.scalar.activation(qex[:], psp[:], AF.Identity,
                                                     bias=bqs_sb[:, p:p + 1],
                                                     scale=0.125)
                                nc.vector.tensor_copy(qr[p][:, sl], qex[:])
                                nc.vector.scalar_tensor_tensor(
                                    out=ql[p][:, sl], in0=qex[:], scalar=1.0,
                                    in1=qr[p][:, sl],
                                    op0=ALU.mult, op1=ALU.subtract)
                            elif which == "k":
                                kex = scr_pool.tile([128, 128], F32, name="qex")
                                nc.scalar.activation(kex[:], psp[:], AF.Identity,
                                                     bias=bks_sb[:, p:p + 1],
                                                     scale=1.0)
                                nc.vector.tensor_copy(kr[p][:, sl], kex[:])
                                nc.vector.scalar_tensor_tensor(
                                    out=kl[p][:, sl], in0=kex[:], scalar=1.0,
                                    in1=kr[p][:, sl],
                                    op0=ALU.mult, op1=ALU.subtract)
                            else:
                                for hp in range(2):
                                    h = p * 2 + hp
                                    dsl = bass.ts(hp, 64)
                                    nc.vector.scalar_tensor_tensor(
                                        out=vh[h][:, st, 0:64],
                                        in0=psp[:, dsl], scalar=1.0,
                                        in1=bvb_sb[:, p, dsl],
                                        op0=ALU.mult, op1=ALU.add)

            # ---- phase 2+3: attention + out-projection ----
            with (
                tc.tile_pool(name="small", bufs=8) as small,
                tc.tile_pool(name="attw", bufs=1) as att_pool,
                tc.tile_pool(name="ctxn", bufs=4) as ctx_pool,
                tc.tile_pool(name="ps_a", bufs=2, space="PSUM") as ps_a,
                tc.tile_pool(name="ps_b", bufs=4, space="PSUM") as ps_b,
                tc.tile_pool(name="ps_m", bufs=2, space="PSUM") as ps_m,
            ):
                for ib in range(IB):
                    isl = bass.ts(ib, 512)
                    for p in range(NPAIR):
                        rows = [bass.ts(0, 64), bass.ts(1, 64)]
                        # A: approximate row max of logits (hi parts only);
                        # the two heads' 64-row matmuls pair in the array
                        for it in range(4):
                            i0 = ib * 512 + it * 128
                            itsl = bass.ds(i0, 128)
                            nm2 = [small.tile([128, 4], F32, name="nm2")
                                   for hp in range(2)]
                            for jh in range(4):
                                jsl = bass.ts(jh, 512)
                                psa = [ps_a.tile([128, 512], F32, name="psa")
                                       for hp in range(2)]
                                for hp in range(2):
                                    nc.tensor.matmul(
                                        psa[hp][:], qr[p][rows[hp], itsl],
                                        kr[p][rows[hp], jsl],
                                        start=True, stop=True)
                                for hp in range(2):
                                    nc.vector.reduce_max(
                                        nm2[hp][:, jh:jh + 1], psa[hp][:],
                                        axis=AX.X)
                            for hp in range(2):
                                nm = small.tile([128, 1], F32, name="nm")
                                nc.vector.reduce_max(nm[:], nm2[hp][:], axis=AX.X,
                                                     negate=True)
                                pst2 = ps_m.tile([128, 128], F32, name="pst2")
                                nc.tensor.transpose(pst2[0:1, :], nm[:],
                                                    ident[:])
                                nc.scalar.activation(
                                    nm_h[p * 2 + hp][0:1, itsl], pst2[0:1, :],
                                    AF.Identity, bias=marg[0:1, 0:1], scale=1.0)
                        # B: shifted scores, 3-term f32r hi/lo, head-paired;
                        # -m applied via paired rank-1 accumulate; then exp
                        attT = [att_pool.tile([128, JT, 512], vdt, name=f"attT{hp}")
                                for hp in range(2)]
                        for jt in range(JT):
                            jsl = bass.ts(jt, 128)
                            psb = [ps_b.tile([128, 512], F32, name="psb")
                                   for hp in range(2)]
                            for hp in range(2):
                                nc.tensor.matmul(psb[hp][:], kr[p][rows[hp], jsl],
                                                 qr[p][rows[hp], isl],
                                                 start=True, stop=False)
                            for hp in range(2):
                                nc.tensor.matmul(psb[hp][:], kr[p][rows[hp], jsl],
                                                 ql[p][rows[hp], isl],
                                                 start=False, stop=False)
                            for hp in range(2):
                                nc.tensor.matmul(psb[hp][:], kl[p][rows[hp], jsl],
                                                 qr[p][rows[hp], isl],
                                                 start=False, stop=False)
                            for hp in range(2):
                                nc.tensor.matmul(psb[hp][:],
                                                 ones_r[0:1, 0:128],
                                                 nm_h[p * 2 + hp][0:1, isl],
                                                 start=False, stop=True)
                            for hp in range(2):
                                nc.scalar.activation(attT[hp][:, jt, :],
                                                     psb[hp][:], AF.Exp)
                        # C: att @ V-hat, normalize, transpose ctx
                        for hp in range(2):
                            h = p * 2 + hp
                            for it in range(4):
                                i0 = ib * 512 + it * 128
                                psc = ps_m.tile([128, 128], F32, name="pst2")
                                for jt in range(JT):
                                    nc.tensor.matmul(
                                        psc[:, 0:65],
                                        attT[hp][:, jt, bass.ts(it, 128)],
                                        vh[h][:, jt, :],
                                        start=(jt == 0), stop=(jt == JT - 1))
                                recip = small.tile([128, 1], F32, name="recip")
                                nc.vector.reciprocal(recip[:], psc[:, 64:65])
                                ctxn = ctx_pool.tile([128, 64], F32, name="ctxn")
                                nc.vector.tensor_scalar_mul(ctxn[:], psc[:, 0:64],
                                                            recip[:])
                                pst2 = ps_m.tile([128, 128], F32, name="pst2")
                                nc.tensor.transpose(pst2[0:64, :], ctxn[:],
                                                    ident[:])
                                nc.vector.tensor_copy(
                                    ctxT[h // 2][bass.ts(h % 2, 64),
                                                 bass.ds(i0, 128)],
                                    pst2[0:64, :])
                    # out-projection for this i-block (float32r)
                    for it in range(4):
                        i0 = ib * 512 + it * 128
                        for eh in range(2):
                            pso = ps_b.tile([128, 512], F32, name="psb")
                            for ct in range(2):
                                nc.tensor.matmul(pso[:],
                                                 ctxT[ct][:, bass.ds(i0, 128)],
                                                 wo_sb[:, ct, bass.ts(eh, 512)],
                                                 start=(ct == 0), stop=(ct == 1))
                            outsb = ctx_pool.tile([128, 512], F32, name="outsb")
                            nc.scalar.copy(outsb[:], pso[:])
                            nc.sync.dma_start(out_p[bass.ds(i0, 128),
                                                    bass.ts(eh, 512)], outsb[:])
    nc.finalize()
    return nc


_NC_CACHE = None


def _get_nc():
    global _NC_CACHE
    if _NC_CACHE is None:
        _NC_CACHE = build_bass()
    return _NC_CACHE


def _prep_core_inputs(inputs, core):
    b, hg = core // 4, core % 4
    h0 = hg * HPC
    q, k, v = inputs["q"], inputs["k"], inputs["v"]
    Wq, Wk, Wv = inputs["Wq"], inputs["Wk"], inputs["Wv"]
    bq, bk, bv = inputs["bq"], inputs["bk"], inputs["bv"]
    Wo = inputs["Wo"]

    def pack_w(W):
        # [NPAIR, ET, 128, 128]: pair p, e-tile t -> [W[h0+2p] | W[h0+2p+1]]
        out = np.empty((NPAIR, ET, 128, 128), np.float32)
        for p in range(NPAIR):
            pair = np.concatenate([W[h0 + 2 * p], W[h0 + 2 * p + 1]], axis=1)
            out[p] = pair.reshape(ET, 128, 128)
        return out

    def pack_bcol(bias, scale):
        out = np.empty((128, NPAIR), np.float32)
        for p in range(NPAIR):
            out[:, p] = np.concatenate(
                [bias[h0 + 2 * p], bias[h0 + 2 * p + 1]]) * scale
        return out

    bvb = np.empty((128, NPAIR, 128), np.float32)
    for p in range(NPAIR):
        bvb[:, p, :] = np.concatenate([bv[h0 + 2 * p], bv[h0 + 2 * p + 1]])[None, :]

    wo_rows = Wo[h0 * DH:(h0 + HPC) * DH, :]  # [256, E]
    vdt_np = mybir.dt.np(ATT_DTYPE)
    return {
        "x_q": np.ascontiguousarray(q[b]),
        "x_k": np.ascontiguousarray(k[b]),
        "x_v": np.ascontiguousarray(v[b]).astype(vdt_np),
        "wq": pack_w(Wq), "wk": pack_w(Wk),
        "wv": pack_w(Wv).astype(vdt_np),
        "bqs": pack_bcol(bq, 0.125), "bks": pack_bcol(bk, 1.0), "bvb": bvb,
        "wo": np.ascontiguousarray(wo_rows.reshape(NPAIR, 128, E)),
    }


def run(inputs, trace=False, **kw):
    inputs = {k: np.asarray(v) for k, v in inputs.items()}
    nc = _get_nc()
    in_maps = [_prep_core_inputs(inputs, c) for c in range(NCORES)]
    res = run_bass_kernel_spmd(nc, in_maps, list(range(NCORES)), trace=trace, **kw)
    bo = inputs["bo"]
    out = np.empty((B, S, E), np.float32)
    for b in range(B):
        acc = res.results[b * 4]["out_p"].astype(np.float32)
        for c in range(b * 4 + 1, b * 4 + 4):
            acc = acc + res.results[c]["out_p"]
        out[b] = acc + bo[None, :]
    return out, res


def kernel(**inputs):
    out, _ = run(inputs)
    return out


# revision 29
# speedup vs baseline: 1.9653x; 1.0917x over previous
"""Multi-head attention TRN2 Bass kernel.

Problem: B=2, S=2048, E=1024, H=16, Dh=64; per-head QKV projection weights,
unmasked softmax(Q K^T / sqrt(Dh)) @ V, concat heads, out-projection.

Sharding: 8 cores = 2 batches x 4 head-groups (4 heads each). Each core
computes its batch/head-group's attention and a partial out-projection
(the rows of Wo belonging to its heads); the host sums the 4 partials per
batch and adds bo.

Numerics: the attention logits here have std ~1200, so softmax is
near-one-hot and argmax flips dominate the error; the Q/K/scores path
needs fp32-level precision. Scores run as a 3-term float32r hi/lo
decomposition (Qr·Kr + Qr·Kl + Ql·Kr) which is fp32-accurate but runs at
1 cycle/row instead of fp32's 4. Heads are packed in pairs on partitions
0:64 / 64:128 so the two heads' 64-row score matmuls run concurrently in
the PE array (row groups). The softmax row-max (from a cheap hi-only
scores pass) is subtracted via an accumulated rank-1 matmul, and the
softmax denominator falls out of an extra ones-column on V. The V path
(v transpose, V projection, att^T, att@V) runs in bf16; the
out-projection in float32r.
"""

import numpy as np

import concourse.bacc as bacc
import concourse.bass as bass
import concourse.mybir as mybir
import concourse.tile as tile
from concourse import masks
from concourse.bass_utils import run_bass_kernel_spmd

F32 = mybir.dt.float32
F32R = mybir.dt.float32r
BF16 = mybir.dt.bfloat16
AX = mybir.AxisListType
AF = mybir.ActivationFunctionType
ALU = mybir.AluOpType

B, S, E, H, DH = 2, 2048, 1024, 16, 64
NCORES = 8
HPC = 4          # heads per core
NPAIR = 2        # head pairs per core
ET = E // 128    # 8 e-tiles
ST = S // 128    # 16 s-tiles
IB = S // 512    # 4 i-blocks
JT = S // 128    # 16 j-tiles
MARGIN = 32.0    # safety margin for the approximate row max

ATT_DTYPE = BF16  # V path dtype


def build_bass():
    nc = bacc.Bacc("TRN2", target_bir_lowering=False, debug=False,
                   num_devices=NCORES)
    vdt = ATT_DTYPE
    x_q = nc.dram_tensor("x_q", [S, E], F32, kind="ExternalInput")
    x_k = nc.dram_tensor("x_k", [S, E], F32, kind="ExternalInput")
    x_v = nc.dram_tensor("x_v", [S, E], vdt, kind="ExternalInput")
    wq = nc.dram_tensor("wq", [NPAIR, ET, 128, 128], F32, kind="ExternalInput")
    wk = nc.dram_tensor("wk", [NPAIR, ET, 128, 128], F32, kind="ExternalInput")
    wv = nc.dram_tensor("wv", [NPAIR, ET, 128, 128], vdt, kind="ExternalInput")
    bqs = nc.dram_tensor("bqs", [128, NPAIR], F32, kind="ExternalInput")
    bks = nc.dram_tensor("bks", [128, NPAIR], F32, kind="ExternalInput")
    bvb = nc.dram_tensor("bvb", [128, NPAIR, 128], F32, kind="ExternalInput")
    wo = nc.dram_tensor("wo", [NPAIR, 128, E], F32, kind="ExternalInput")
    out_p = nc.dram_tensor("out_p", [S, E], F32, kind="ExternalOutput")

    with tile.TileContext(nc) as tc:
        with (
            tc.tile_pool(name="const", bufs=1) as const_pool,
            tc.tile_pool(name="persist", bufs=1) as persist,
        ):
            ident = const_pool.tile([128, 128], F32, name="ident")
            masks.make_identity(nc, ident[:])
            ident_v = const_pool.tile([128, 128], vdt, name="ident_v")
            masks.make_identity(nc, ident_v[:])
            marg = const_pool.tile([128, 1], F32, name="marg")
            nc.gpsimd.memset(marg[:], -MARGIN)
            # all-ones f32r tile for the rank-1 (-m) accumulate; rounded via
            # a copy so the f32r matmult verifier accepts it
            ones_st = const_pool.tile([128, 128], F32, name="ones_st")
            nc.gpsimd.memset(ones_st[:], 1.0)
            ones_r = const_pool.tile([128, 128], BF16, name="ones_r")
            nc.vector.tensor_copy(ones_r[:], ones_st[:])

            bqs_sb = const_pool.tile([128, NPAIR], F32, name="bqs")
            nc.sync.dma_start(bqs_sb[:], bqs[:])
            bks_sb = const_pool.tile([128, NPAIR], F32, name="bks")
            nc.sync.dma_start(bks_sb[:], bks[:])
            bvb_sb = const_pool.tile([128, NPAIR, 128], F32, name="bvb")
            nc.sync.dma_start(bvb_sb[:], bvb[:])
            # out-projection runs in float32r; round via copy
            wo_st = const_pool.tile([128, NPAIR, E], F32, name="wo_st")
            nc.sync.dma_start(wo_st[:], wo.rearrange("c p e -> p c e"))
            wo_sb = const_pool.tile([128, NPAIR, E], F32R, name="wo")
            nc.vector.tensor_copy(wo_sb[:], wo_st[:])

            # persistent per-pair packed tensors (rows 0:64 = even head,
            # rows 64:128 = odd head of the pair)
            qr = [persist.tile([128, S], BF16, name=f"qr{p}") for p in range(NPAIR)]
            ql = [persist.tile([128, S], BF16, name=f"ql{p}") for p in range(NPAIR)]
            kr = [persist.tile([128, S], BF16, name=f"kr{p}") for p in range(NPAIR)]
            kl = [persist.tile([128, S], BF16, name=f"kl{p}") for p in range(NPAIR)]
            # -(rowmax)-MARGIN per pair: even head on partition 0, odd head
            # on partition 64 so the two rank-1 (-m) matmuls pair in the
            # array's row groups
            nm_p = [persist.tile([128, S], BF16, name=f"nmp{p}")
                    for p in range(NPAIR)]
            vh = [persist.tile([128, JT, 65], vdt, name=f"vh{h}")
                  for h in range(HPC)]
            ctxT = [persist.tile([128, S], F32R, name=f"ctxT{c}") for c in range(2)]

            for h in range(HPC):
                nc.gpsimd.memset(vh[h][:, :, 64:65], 1.0)

            # ---- phase 1: load, transpose, project ----
            with (
                tc.tile_pool(name="stage", bufs=3) as stage_pool,
                tc.tile_pool(name="xt", bufs=3) as xt_pool,
                tc.tile_pool(name="wght", bufs=1) as w_pool,
                tc.tile_pool(name="scr", bufs=3) as scr_pool,
                tc.tile_pool(name="ps_tin", bufs=3, space="PSUM") as ps_tin,
                tc.tile_pool(name="ps_proj", bufs=4, space="PSUM") as ps_proj,
            ):
                for x_dram, w_dram, which in ((x_q, wq, "q"), (x_k, wk, "k"),
                                              (x_v, wv, "v")):
                    dt_in = vdt if which == "v" else F32
                    id_in = ident_v if which == "v" else ident
                    w_sb = w_pool.tile([128, NPAIR, ET, 128], dt_in, name="w_in")
                    nc.sync.dma_start(w_sb[:], w_dram.rearrange("p t e d -> e p t d"))
                    for st in range(ST):
                        sl = bass.ts(st, 128)
                        stage = stage_pool.tile([128, E], dt_in, name="stage")
                        nc.sync.dma_start(stage[:], x_dram[sl, :])
                        xt = xt_pool.tile([128, ET, 128], dt_in, name="xt")
                        for et in range(ET):
                            pst = ps_tin.tile([128, 128], dt_in, name="pst")
                            nc.tensor.transpose(pst[:], stage[:, bass.ts(et, 128)],
                                                id_in[:])
                            nc.vector.tensor_copy(xt[:, et, :], pst[:])
                        for p in range(NPAIR):
                            psp = ps_proj.tile([128, 128], F32, name="psp")
                            for et in range(ET):
                                if which == "v":
                                    lhsT, rhs = xt[:, et, :], w_sb[:, p, et, :]
                                else:
                                    lhsT, rhs = w_sb[:, p, et, :], xt[:, et, :]
                                nc.tensor.matmul(psp[:], lhsT, rhs,
                                                 start=(et == 0), stop=(et == ET - 1))
                            if which == "q":
                                qex = scr_pool.tile([128, 128], F32, name="qex")
                                nc.scalar.activation(qex[:], psp[:], AF.Identity,
                                                     bias=bqs_sb[:, p:p + 1],
                                                     scale=0.125)
                                nc.vector.tensor_copy(qr[p][:, sl], qex[:])
                                nc.vector.scalar_tensor_tensor(
                                    out=ql[p][:, sl], in0=qex[:], scalar=1.0,
                                    in1=qr[p][:, sl],
                                    op0=ALU.mult, op1=ALU.subtract)
                            elif which == "k":
                                kex = scr_pool.tile([128, 128], F32, name="qex")
                                nc.scalar.activation(kex[:], psp[:], AF.Identity,
                                                     bias=bks_sb[:, p:p + 1],
                                                     scale=1.0)
                                nc.vector.tensor_copy(kr[p][:, sl], kex[:])
                                nc.vector.scalar_tensor_tensor(
                                    out=kl[p][:, sl], in0=kex[:], scalar=1.0,
                                    in1=kr[p][:, sl],
                                    op0=ALU.mult, op1=ALU.subtract)
                            else:
                                for hp in range(2):
                                    h = p * 2 + hp
                                    dsl = bass.ts(hp, 64)
                                    nc.vector.scalar_tensor_tensor(
                                        out=vh[h][:, st, 0:64],
                                        in0=psp[:, dsl], scalar=1.0,
                                        in1=bvb_sb[:, p, dsl],
                                        op0=ALU.mult, op1=ALU.add)

            # ---- phase 2+3: attention + out-projection ----
            with (
                tc.tile_pool(name="small", bufs=8) as small,
                tc.tile_pool(name="attw", bufs=1) as att_pool,
                tc.tile_pool(name="ctxn", bufs=4) as ctx_pool,
                tc.tile_pool(name="ps_a", bufs=2, space="PSUM") as ps_a,
                tc.tile_pool(name="ps_b", bufs=4, space="PSUM") as ps_b,
                tc.tile_pool(name="ps_m", bufs=2, space="PSUM") as ps_m,
            ):
                for ib in range(IB):
                    isl = bass.ts(ib, 512)
                    for p in range(NPAIR):
                        rows = [bass.ts(0, 64), bass.ts(1, 64)]
                        # A: approximate row max of logits (hi parts only);
                        # the two heads' 64-row matmuls pair in the array
                        for it in range(4):
                            i0 = ib * 512 + it * 128
                            itsl = bass.ds(i0, 128)
                            nm2 = [small.tile([128, 4], F32, name="nm2")
                                   for hp in range(2)]
                            for jh in range(4):
                                jsl = bass.ts(jh, 512)
                                psa = [ps_a.tile([128, 512], F32, name="psa")
                                       for hp in range(2)]
                                for hp in range(2):
                                    nc.tensor.matmul(
                                        psa[hp][:], qr[p][rows[hp], itsl],
                                        kr[p][rows[hp], jsl],
                                        start=True, stop=True)
                                for hp in range(2):
                                    nc.vector.reduce_max(
                                        nm2[hp][:, jh:jh + 1], psa[hp][:],
                                        axis=AX.X)
                            for hp in range(2):
                                nm = small.tile([128, 1], F32, name="nm")
                                nc.vector.reduce_max(nm[:], nm2[hp][:], axis=AX.X,
                                                     negate=True)
                                pst2 = ps_m.tile([128, 128], F32, name="pst2")
                                nc.tensor.transpose(pst2[0:1, :], nm[:],
                                                    ident[:])
                                if hp == 0:
                                    nc.scalar.activation(
                                        nm_p[p][0:1, itsl], pst2[0:1, :],
                                        AF.Identity, bias=marg[0:1, 0:1], scale=1.0)
                                else:
                                    # engines cannot shift partitions; bounce
                                    # through a scratch row + tiny SBUF DMA
                                    nms = small.tile([1, 128], BF16, name="nms")
                                    nc.scalar.activation(
                                        nms[:], pst2[0:1, :],
                                        AF.Identity, bias=marg[0:1, 0:1], scale=1.0)
                                    nc.sync.dma_start(nm_p[p][64:65, itsl], nms[:])
                        # B: shifted scores, 3-term f32r hi/lo, head-paired;
                        # -m applied via paired rank-1 accumulate; then exp
                        attT = [att_pool.tile([128, JT, 512], vdt, name=f"attT{hp}")
                                for hp in range(2)]
                        for jt in range(JT):
                            jsl = bass.ts(jt, 128)
                            psb = [ps_b.tile([128, 512], F32, name="psb")
                                   for hp in range(2)]
                            for hp in range(2):
                                nc.tensor.matmul(psb[hp][:], kr[p][rows[hp], jsl],
                                                 qr[p][rows[hp], isl],
                                                 start=True, stop=False)
                            for hp in range(2):
                                nc.tensor.matmul(psb[hp][:], kr[p][rows[hp], jsl],
                                                 ql[p][rows[hp], isl],
                                                 start=False, stop=False)
                            for hp in range(2):
                                nc.tensor.matmul(psb[hp][:], kl[p][rows[hp], jsl],
                                                 qr[p][rows[hp], isl],
                                                 start=False, stop=False)
                            for hp in range(2):
                                r0 = hp * 64
                                nc.tensor.matmul(psb[hp][:],
                                                 ones_r[r0:r0 + 1, 0:128],
                                                 nm_p[p][r0:r0 + 1, isl],
                                                 start=False, stop=True)
                            for hp in range(2):
                                nc.scalar.activation(attT[hp][:, jt, :],
                                                     psb[hp][:], AF.Exp)
                        # C: att @ V-hat, normalize, transpose ctx
                        for hp in range(2):
                            h = p * 2 + hp
                            for it in range(4):
                                i0 = ib * 512 + it * 128
                                psc = ps_m.tile([128, 128], F32, name="pst2")
                                for jt in range(JT):
                                    nc.tensor.matmul(
                                        psc[:, 0:65],
                                        attT[hp][:, jt, bass.ts(it, 128)],
                                        vh[h][:, jt, :],
                                        start=(jt == 0), stop=(jt == JT - 1))
                                recip = small.tile([128, 1], F32, name="recip")
                                nc.vector.reciprocal(recip[:], psc[:, 64:65])
                                ctxn = ctx_pool.tile([128, 64], F32, name="ctxn")
                                nc.vector.tensor_scalar_mul(ctxn[:], psc[:, 0:64],
                                                            recip[:])
                                pst2 = ps_m.tile([128, 128], F32, name="pst2")
                                nc.tensor.transpose(pst2[0:64, :], ctxn[:],
                                                    ident[:])
                                nc.vector.tensor_copy(
                                    ctxT[h // 2][bass.ts(h % 2, 64),
                                                 bass.ds(i0, 128)],
                                    pst2[0:64, :])
                    # out-projection for this i-block (float32r)
                    for it in range(4):
                        i0 = ib * 512 + it * 128
                        for eh in range(2):
                            pso = ps_b.tile([128, 512], F32, name="psb")
                            for ct in range(2):
                                nc.tensor.matmul(pso[:],
                                                 ctxT[ct][:, bass.ds(i0, 128)],
                                                 wo_sb[:, ct, bass.ts(eh, 512)],
                                                 start=(ct == 0), stop=(ct == 1))
                            outsb = ctx_pool.tile([128, 512], F32, name="outsb")
                            nc.scalar.copy(outsb[:], pso[:])
                            nc.sync.dma_start(out_p[bass.ds(i0, 128),
                                                    bass.ts(eh, 512)], outsb[:])
    nc.finalize()
    return nc


_NC_CACHE = None


def _get_nc():
    global _NC_CACHE
    if _NC_CACHE is None:
        _NC_CACHE = build_bass()
    return _NC_CACHE


def _prep_core_inputs(inputs, core):
    b, hg = core // 4, core % 4
    h0 = hg * HPC
    q, k, v = inputs["q"], inputs["k"], inputs["v"]
    Wq, Wk, Wv = inputs["Wq"], inputs["Wk"], inputs["Wv"]
    bq, bk, bv = inputs["bq"], inputs["bk"], inputs["bv"]
    Wo = inputs["Wo"]

    def pack_w(W):
        # [NPAIR, ET, 128, 128]: pair p, e-tile t -> [W[h0+2p] | W[h0+2p+1]]
        out = np.empty((NPAIR, ET, 128, 128), np.float32)
        for p in range(NPAIR):
            pair = np.concatenate([W[h0 + 2 * p], W[h0 + 2 * p + 1]], axis=1)
            out[p] = pair.reshape(ET, 128, 128)
        return out

    def pack_bcol(bias, scale):
        out = np.empty((128, NPAIR), np.float32)
        for p in range(NPAIR):
            out[:, p] = np.concatenate(
                [bias[h0 + 2 * p], bias[h0 + 2 * p + 1]]) * scale
        return out

    bvb = np.empty((128, NPAIR, 128), np.float32)
    for p in range(NPAIR):
        bvb[:, p, :] = np.concatenate([bv[h0 + 2 * p], bv[h0 + 2 * p + 1]])[None, :]

    wo_rows = Wo[h0 * DH:(h0 + HPC) * DH, :]  # [256, E]
    vdt_np = mybir.dt.np(ATT_DTYPE)
    return {
        "x_q": np.ascontiguousarray(q[b]),
        "x_k": np.ascontiguousarray(k[b]),
        "x_v": np.ascontiguousarray(v[b]).astype(vdt_np),
        "wq": pack_w(Wq), "wk": pack_w(Wk),
        "wv": pack_w(Wv).astype(vdt_np),
        "bqs": pack_bcol(bq, 0.125), "bks": pack_bcol(bk, 1.0), "bvb": bvb,
        "wo": np.ascontiguousarray(wo_rows.reshape(NPAIR, 128, E)),
    }


def run(inputs, trace=False, **kw):
    inputs = {k: np.asarray(v) for k, v in inputs.items()}
    nc = _get_nc()
    in_maps = [_prep_core_inputs(inputs, c) for c in range(NCORES)]
    res = run_bass_kernel_spmd(nc, in_maps, list(range(NCORES)), trace=trace, **kw)
    bo = inputs["bo"]
    out = np.empty((B, S, E), np.float32)
    for b in range(B):
        acc = res.results[b * 4]["out_p"].astype(np.float32)
        for c in range(b * 4 + 1, b * 4 + 4):
            acc = acc + res.results[c]["out_p"]
        out[b] = acc + bo[None, :]
    return out, res


def kernel(**inputs):
    out, _ = run(inputs)
    return out


# revision 31
# speedup vs baseline: 2.0300x; 1.0329x over previous
"""Multi-head attention TRN2 Bass kernel.

Problem: B=2, S=2048, E=1024, H=16, Dh=64; per-head QKV projection weights,
unmasked softmax(Q K^T / sqrt(Dh)) @ V, concat heads, out-projection.

Sharding: 8 cores = 2 batches x 4 head-groups (4 heads each). Each core
computes its batch/head-group's attention and a partial out-projection
(the rows of Wo belonging to its heads); the host sums the 4 partials per
batch and adds bo.

Numerics: the attention logits here have std ~1200, so softmax is
near-one-hot and argmax flips dominate the error; the Q/K/scores path
needs fp32-level precision. Q/K projections run in fp32; scores run as a
3-term bf16 hi/lo decomposition (Qhi·Khi + Qhi·Klo + Qlo·Khi, f32 PSUM
accumulate) which keeps fp32-level logit accuracy (dropped lo·lo term is
~2^-18 relative) while running at 1 cycle/row at any clock state — this
matters because the firmware power limiter holds the PE at K=4/8
(1.2 GHz) for most of a sustained matmul stream, where fp32 is 4 cyc/row
and even float32r degrades to 2. Heads are packed in pairs on partitions
0:64 / 64:128 so the two heads' 64-row score matmuls (and the rank-1
row-max subtraction accumulates) run concurrently in the PE array's row
groups. The softmax row-max comes from a cheap hi-only scores pass in the
[i,j] layout (free-dim reduce; softmax is shift-invariant so a ±few-units
max with a safety margin is fine), and the softmax denominator falls out
of an extra ones-column on V. The V path (v transpose, V projection,
att^T, att@V) runs in bf16; the out-projection in float32r.
"""

import numpy as np

import concourse.bacc as bacc
import concourse.bass as bass
import concourse.mybir as mybir
import concourse.tile as tile
from concourse import masks
from concourse.bass_utils import run_bass_kernel_spmd

F32 = mybir.dt.float32
F32R = mybir.dt.float32r
BF16 = mybir.dt.bfloat16
AX = mybir.AxisListType
AF = mybir.ActivationFunctionType
ALU = mybir.AluOpType

B, S, E, H, DH = 2, 2048, 1024, 16, 64
NCORES = 8
HPC = 4          # heads per core
NPAIR = 2        # head pairs per core
ET = E // 128    # 8 e-tiles
ST = S // 128    # 16 s-tiles
IB = S // 512    # 4 i-blocks
JT = S // 128    # 16 j-tiles
MARGIN = 32.0    # safety margin for the approximate row max

ATT_DTYPE = BF16  # V path dtype


def build_bass():
    nc = bacc.Bacc("TRN2", target_bir_lowering=False, debug=False,
                   num_devices=NCORES)
    vdt = ATT_DTYPE
    x_q = nc.dram_tensor("x_q", [S, E], F32, kind="ExternalInput")
    x_k = nc.dram_tensor("x_k", [S, E], F32, kind="ExternalInput")
    x_v = nc.dram_tensor("x_v", [S, E], vdt, kind="ExternalInput")
    wq = nc.dram_tensor("wq", [NPAIR, ET, 128, 128], F32, kind="ExternalInput")
    wk = nc.dram_tensor("wk", [NPAIR, ET, 128, 128], F32, kind="ExternalInput")
    wv = nc.dram_tensor("wv", [NPAIR, ET, 128, 128], vdt, kind="ExternalInput")
    bqs = nc.dram_tensor("bqs", [128, NPAIR], F32, kind="ExternalInput")
    bks = nc.dram_tensor("bks", [128, NPAIR], F32, kind="ExternalInput")
    bvb = nc.dram_tensor("bvb", [128, NPAIR, 128], F32, kind="ExternalInput")
    wo = nc.dram_tensor("wo", [NPAIR, 128, E], F32, kind="ExternalInput")
    out_p = nc.dram_tensor("out_p", [S, E], F32, kind="ExternalOutput")

    with tile.TileContext(nc) as tc:
        with (
            tc.tile_pool(name="const", bufs=1) as const_pool,
            tc.tile_pool(name="persist", bufs=1) as persist,
        ):
            ident = const_pool.tile([128, 128], F32, name="ident")
            masks.make_identity(nc, ident[:])
            ident_v = const_pool.tile([128, 128], vdt, name="ident_v")
            masks.make_identity(nc, ident_v[:])
            marg = const_pool.tile([128, 1], F32, name="marg")
            nc.gpsimd.memset(marg[:], -MARGIN)
            # all-ones f32r tile for the rank-1 (-m) accumulate; rounded via
            # a copy so the f32r matmult verifier accepts it
            ones_st = const_pool.tile([128, 128], F32, name="ones_st")
            nc.gpsimd.memset(ones_st[:], 1.0)
            ones_r = const_pool.tile([128, 128], BF16, name="ones_r")
            nc.vector.tensor_copy(ones_r[:], ones_st[:])

            bqs_sb = const_pool.tile([128, NPAIR], F32, name="bqs")
            nc.sync.dma_start(bqs_sb[:], bqs[:])
            bks_sb = const_pool.tile([128, NPAIR], F32, name="bks")
            nc.sync.dma_start(bks_sb[:], bks[:])
            bvb_sb = const_pool.tile([128, NPAIR, 128], F32, name="bvb")
            nc.sync.dma_start(bvb_sb[:], bvb[:])
            # out-projection runs in float32r; round via copy
            wo_st = const_pool.tile([128, NPAIR, E], F32, name="wo_st")
            nc.sync.dma_start(wo_st[:], wo.rearrange("c p e -> p c e"))
            wo_sb = const_pool.tile([128, NPAIR, E], BF16, name="wo")
            nc.vector.tensor_copy(wo_sb[:], wo_st[:])

            # persistent per-pair packed tensors (rows 0:64 = even head,
            # rows 64:128 = odd head of the pair)
            qr = [persist.tile([128, S], BF16, name=f"qr{p}") for p in range(NPAIR)]
            ql = [persist.tile([128, S], BF16, name=f"ql{p}") for p in range(NPAIR)]
            kr = [persist.tile([128, S], BF16, name=f"kr{p}") for p in range(NPAIR)]
            kl = [persist.tile([128, S], BF16, name=f"kl{p}") for p in range(NPAIR)]
            # -(rowmax)-MARGIN per pair: even head on partition 0, odd head
            # on partition 64 so the two rank-1 (-m) matmuls pair in the
            # array's row groups
            nm_p = [persist.tile([128, S], BF16, name=f"nmp{p}")
                    for p in range(NPAIR)]
            vh = [persist.tile([128, JT, 65], vdt, name=f"vh{h}")
                  for h in range(HPC)]
            ctxT = [persist.tile([128, S], BF16, name=f"ctxT{c}") for c in range(2)]

            for h in range(HPC):
                nc.gpsimd.memset(vh[h][:, :, 64:65], 1.0)

            # ---- phase 1: load, transpose, project ----
            with (
                tc.tile_pool(name="stage", bufs=3) as stage_pool,
                tc.tile_pool(name="xt", bufs=3) as xt_pool,
                tc.tile_pool(name="wght", bufs=1) as w_pool,
                tc.tile_pool(name="scr", bufs=3) as scr_pool,
                tc.tile_pool(name="ps_tin", bufs=3, space="PSUM") as ps_tin,
                tc.tile_pool(name="ps_proj", bufs=4, space="PSUM") as ps_proj,
            ):
                for x_dram, w_dram, which in ((x_q, wq, "q"), (x_k, wk, "k"),
                                              (x_v, wv, "v")):
                    dt_in = vdt if which == "v" else F32
                    id_in = ident_v if which == "v" else ident
                    w_sb = w_pool.tile([128, NPAIR, ET, 128], dt_in, name="w_in")
                    nc.sync.dma_start(w_sb[:], w_dram.rearrange("p t e d -> e p t d"))
                    for st in range(ST):
                        sl = bass.ts(st, 128)
                        stage = stage_pool.tile([128, E], dt_in, name="stage")
                        nc.sync.dma_start(stage[:], x_dram[sl, :])
                        xt = xt_pool.tile([128, ET, 128], dt_in, name="xt")
                        for et in range(ET):
                            pst = ps_tin.tile([128, 128], dt_in, name="pst")
                            nc.tensor.transpose(pst[:], stage[:, bass.ts(et, 128)],
                                                id_in[:])
                            nc.vector.tensor_copy(xt[:, et, :], pst[:])
                        for p in range(NPAIR):
                            psp = ps_proj.tile([128, 128], F32, name="psp")
                            for et in range(ET):
                                if which == "v":
                                    lhsT, rhs = xt[:, et, :], w_sb[:, p, et, :]
                                else:
                                    lhsT, rhs = w_sb[:, p, et, :], xt[:, et, :]
                                nc.tensor.matmul(psp[:], lhsT, rhs,
                                                 start=(et == 0), stop=(et == ET - 1))
                            if which == "q":
                                qex = scr_pool.tile([128, 128], F32, name="qex")
                                nc.scalar.activation(qex[:], psp[:], AF.Identity,
                                                     bias=bqs_sb[:, p:p + 1],
                                                     scale=0.125)
                                nc.vector.tensor_copy(qr[p][:, sl], qex[:])
                                nc.vector.scalar_tensor_tensor(
                                    out=ql[p][:, sl], in0=qex[:], scalar=1.0,
                                    in1=qr[p][:, sl],
                                    op0=ALU.mult, op1=ALU.subtract)
                            elif which == "k":
                                kex = scr_pool.tile([128, 128], F32, name="qex")
                                nc.scalar.activation(kex[:], psp[:], AF.Identity,
                                                     bias=bks_sb[:, p:p + 1],
                                                     scale=1.0)
                                nc.vector.tensor_copy(kr[p][:, sl], kex[:])
                                nc.vector.scalar_tensor_tensor(
                                    out=kl[p][:, sl], in0=kex[:], scalar=1.0,
                                    in1=kr[p][:, sl],
                                    op0=ALU.mult, op1=ALU.subtract)
                            else:
                                for hp in range(2):
                                    h = p * 2 + hp
                                    dsl = bass.ts(hp, 64)
                                    nc.vector.scalar_tensor_tensor(
                                        out=vh[h][:, st, 0:64],
                                        in0=psp[:, dsl], scalar=1.0,
                                        in1=bvb_sb[:, p, dsl],
                                        op0=ALU.mult, op1=ALU.add)

            # ---- phase 2+3: attention + out-projection ----
            with (
                tc.tile_pool(name="small", bufs=8) as small,
                tc.tile_pool(name="attw", bufs=1) as att_pool,
                tc.tile_pool(name="ctxn", bufs=4) as ctx_pool,
                tc.tile_pool(name="ps_a", bufs=2, space="PSUM") as ps_a,
                tc.tile_pool(name="ps_b", bufs=4, space="PSUM") as ps_b,
                tc.tile_pool(name="ps_m", bufs=2, space="PSUM") as ps_m,
            ):
                for ib in range(IB):
                    isl = bass.ts(ib, 512)
                    for p in range(NPAIR):
                        rows = [bass.ts(0, 64), bass.ts(1, 64)]
                        # A: approximate row max of logits (hi parts only);
                        # the two heads' 64-row matmuls pair in the array
                        for it in range(4):
                            i0 = ib * 512 + it * 128
                            itsl = bass.ds(i0, 128)
                            nm2 = [small.tile([128, 4], F32, name="nm2")
                                   for hp in range(2)]
                            for jh in range(4):
                                jsl = bass.ts(jh, 512)
                                psa = [ps_a.tile([128, 512], F32, name="psa")
                                       for hp in range(2)]
                                for hp in range(2):
                                    nc.tensor.matmul(
                                        psa[hp][:], qr[p][rows[hp], itsl],
                                        kr[p][rows[hp], jsl],
                                        start=True, stop=True)
                                for hp in range(2):
                                    nc.vector.reduce_max(
                                        nm2[hp][:, jh:jh + 1], psa[hp][:],
                                        axis=AX.X)
                            for hp in range(2):
                                nm = small.tile([128, 1], F32, name="nm")
                                nc.vector.reduce_max(nm[:], nm2[hp][:], axis=AX.X,
                                                     negate=True)
                                pst2 = ps_m.tile([128, 128], F32, name="pst2")
                                nc.tensor.transpose(pst2[0:1, :], nm[:],
                                                    ident[:])
                                if hp == 0:
                                    nc.scalar.activation(
                                        nm_p[p][0:1, itsl], pst2[0:1, :],
                                        AF.Identity, bias=marg[0:1, 0:1], scale=1.0)
                                else:
                                    # engines cannot shift partitions; bounce
                                    # through a scratch row + tiny SBUF DMA
                                    nms = small.tile([1, 128], BF16, name="nms")
                                    nc.scalar.activation(
                                        nms[:], pst2[0:1, :],
                                        AF.Identity, bias=marg[0:1, 0:1], scale=1.0)
                                    nc.sync.dma_start(nm_p[p][64:65, itsl], nms[:])
                        # B: shifted scores, 3-term f32r hi/lo, head-paired;
                        # -m applied via paired rank-1 accumulate; then exp
                        attT = [att_pool.tile([128, JT, 512], vdt, name=f"attT{hp}")
                                for hp in range(2)]
                        for jt in range(JT):
                            jsl = bass.ts(jt, 128)
                            psb = [ps_b.tile([128, 512], F32, name="psb")
                                   for hp in range(2)]
                            for hp in range(2):
                                nc.tensor.matmul(psb[hp][:], kr[p][rows[hp], jsl],
                                                 qr[p][rows[hp], isl],
                                                 start=True, stop=False)
                            for hp in range(2):
                                nc.tensor.matmul(psb[hp][:], kr[p][rows[hp], jsl],
                                                 ql[p][rows[hp], isl],
                                                 start=False, stop=False)
                            for hp in range(2):
                                nc.tensor.matmul(psb[hp][:], kl[p][rows[hp], jsl],
                                                 qr[p][rows[hp], isl],
                                                 start=False, stop=False)
                            for hp in range(2):
                                r0 = hp * 64
                                nc.tensor.matmul(psb[hp][:],
                                                 ones_r[r0:r0 + 1, 0:128],
                                                 nm_p[p][r0:r0 + 1, isl],
                                                 start=False, stop=True)
                            for hp in range(2):
                                nc.scalar.activation(attT[hp][:, jt, :],
                                                     psb[hp][:], AF.Exp)
                        # C: att @ V-hat, normalize, transpose ctx
                        for hp in range(2):
                            h = p * 2 + hp
                            for it in range(4):
                                i0 = ib * 512 + it * 128
                                psc = ps_m.tile([128, 128], F32, name="pst2")
                                for jt in range(JT):
                                    nc.tensor.matmul(
                                        psc[:, 0:65],
                                        attT[hp][:, jt, bass.ts(it, 128)],
                                        vh[h][:, jt, :],
                                        start=(jt == 0), stop=(jt == JT - 1))
                                recip = small.tile([128, 1], F32, name="recip")
                                nc.vector.reciprocal(recip[:], psc[:, 64:65])
                                ctxn = ctx_pool.tile([128, 64], BF16, name="ctxn")
                                nc.vector.tensor_scalar_mul(ctxn[:], psc[:, 0:64],
                                                            recip[:])
                                pst2 = ps_m.tile([128, 128], BF16, name="pst2")
                                nc.tensor.transpose(pst2[0:64, :], ctxn[:],
                                                    ident_v[:])
                                nc.vector.tensor_copy(
                                    ctxT[h // 2][bass.ts(h % 2, 64),
                                                 bass.ds(i0, 128)],
                                    pst2[0:64, :])
                    # out-projection for this i-block (float32r)
                    for it in range(4):
                        i0 = ib * 512 + it * 128
                        for eh in range(2):
                            pso = ps_b.tile([128, 512], F32, name="psb")
                            for ct in range(2):
                                nc.tensor.matmul(pso[:],
                                                 ctxT[ct][:, bass.ds(i0, 128)],
                                                 wo_sb[:, ct, bass.ts(eh, 512)],
                                                 start=(ct == 0), stop=(ct == 1))
                            outsb = ctx_pool.tile([128, 512], F32, name="outsb")
                            nc.scalar.copy(outsb[:], pso[:])
                            nc.sync.dma_start(out_p[bass.ds(i0, 128),
                                                    bass.ts(eh, 512)], outsb[:])
    nc.finalize()
    return nc


_NC_CACHE = None


def _get_nc():
    global _NC_CACHE
    if _NC_CACHE is None:
        _NC_CACHE = build_bass()
    return _NC_CACHE


def _prep_core_inputs(inputs, core):
    b, hg = core // 4, core % 4
    h0 = hg * HPC
    q, k, v = inputs["q"], inputs["k"], inputs["v"]
    Wq, Wk, Wv = inputs["Wq"], inputs["Wk"], inputs["Wv"]
    bq, bk, bv = inputs["bq"], inputs["bk"], inputs["bv"]
    Wo = inputs["Wo"]

    def pack_w(W):
        # [NPAIR, ET, 128, 128]: pair p, e-tile t -> [W[h0+2p] | W[h0+2p+1]]
        out = np.empty((NPAIR, ET, 128, 128), np.float32)
        for p in range(NPAIR):
            pair = np.concatenate([W[h0 + 2 * p], W[h0 + 2 * p + 1]], axis=1)
            out[p] = pair.reshape(ET, 128, 128)
        return out

    def pack_bcol(bias, scale):
        out = np.empty((128, NPAIR), np.float32)
        for p in range(NPAIR):
            out[:, p] = np.concatenate(
                [bias[h0 + 2 * p], bias[h0 + 2 * p + 1]]) * scale
        return out

    bvb = np.empty((128, NPAIR, 128), np.float32)
    for p in range(NPAIR):
        bvb[:, p, :] = np.concatenate([bv[h0 + 2 * p], bv[h0 + 2 * p + 1]])[None, :]

    wo_rows = Wo[h0 * DH:(h0 + HPC) * DH, :]  # [256, E]
    vdt_np = mybir.dt.np(ATT_DTYPE)
    return {
        "x_q": np.ascontiguousarray(q[b]),
        "x_k": np.ascontiguousarray(k[b]),
        "x_v": np.ascontiguousarray(v[b]).astype(vdt_np),
        "wq": pack_w(Wq), "wk": pack_w(Wk),
        "wv": pack_w(Wv).astype(vdt_np),
        "bqs": pack_bcol(bq, 0.125), "bks": pack_bcol(bk, 1.0), "bvb": bvb,
        "wo": np.ascontiguousarray(wo_rows.reshape(NPAIR, 128, E)),
    }


def run(inputs, trace=False, **kw):
    inputs = {k: np.asarray(v) for k, v in inputs.items()}
    nc = _get_nc()
    in_maps = [_prep_core_inputs(inputs, c) for c in range(NCORES)]
    res = run_bass_kernel_spmd(nc, in_maps, list(range(NCORES)), trace=trace, **kw)
    bo = inputs["bo"]
    out = np.empty((B, S, E), np.float32)
    for b in range(B):
        acc = res.results[b * 4]["out_p"].astype(np.float32)
        for c in range(b * 4 + 1, b * 4 + 4):
            acc = acc + res.results[c]["out_p"]
        out[b] = acc + bo[None, :]
    return out, res


def kernel(**inputs):
    out, _ = run(inputs)
    return out


# revision 35
# speedup vs baseline: 2.0591x; 1.0143x over previous
"""Multi-head attention TRN2 Bass kernel.

Problem: B=2, S=2048, E=1024, H=16, Dh=64; per-head QKV projection weights,
unmasked softmax(Q K^T / sqrt(Dh)) @ V, concat heads, out-projection.

Sharding: 8 cores = 2 batches x 4 head-groups (4 heads each). Each core
computes its batch/head-group's attention and a partial out-projection
(the rows of Wo belonging to its heads); the host sums the 4 partials per
batch and adds bo.

Numerics: the attention logits here have std ~1200, so softmax is
near-one-hot and argmax flips dominate the error; the Q/K/scores path
needs fp32-level precision. Q/K projections run in fp32; scores run as a
3-term bf16 hi/lo decomposition (Qhi·Khi + Qhi·Klo + Qlo·Khi, f32 PSUM
accumulate) which keeps fp32-level logit accuracy (dropped lo·lo term is
~2^-18 relative) while running at 1 cycle/row at any clock state — this
matters because the firmware power limiter holds the PE at K=4/8
(1.2 GHz) for most of a sustained matmul stream, where fp32 is 4 cyc/row
and even float32r degrades to 2. Heads are packed in pairs on partitions
0:64 / 64:128 so the two heads' 64-row score matmuls (and the rank-1
row-max subtraction accumulates) run concurrently in the PE array's row
groups. The softmax row-max comes from a cheap hi-only scores pass in the
[i,j] layout (free-dim reduce; softmax is shift-invariant so a ±few-units
max with a safety margin is fine), and the softmax denominator falls out
of an extra ones-column on V. The V path (v transpose, V projection,
att^T, att@V) and the ctx/out-projection chain run in bf16.
"""

import numpy as np

import concourse.bacc as bacc
import concourse.bass as bass
import concourse.mybir as mybir
import concourse.tile as tile
from concourse import masks
from concourse.bass_utils import run_bass_kernel_spmd

F32 = mybir.dt.float32
F32R = mybir.dt.float32r
BF16 = mybir.dt.bfloat16
AX = mybir.AxisListType
AF = mybir.ActivationFunctionType
ALU = mybir.AluOpType

B, S, E, H, DH = 2, 2048, 1024, 16, 64
NCORES = 8
HPC = 4          # heads per core
NPAIR = 2        # head pairs per core
ET = E // 128    # 8 e-tiles
ST = S // 128    # 16 s-tiles
IB = S // 512    # 4 i-blocks
JT = S // 128    # 16 j-tiles
MARGIN = 32.0    # safety margin for the approximate row max

ATT_DTYPE = BF16  # V path dtype


def build_bass():
    nc = bacc.Bacc("TRN2", target_bir_lowering=False, debug=False,
                   num_devices=NCORES)
    vdt = ATT_DTYPE
    x_q = nc.dram_tensor("x_q", [S, E], F32, kind="ExternalInput")
    x_k = nc.dram_tensor("x_k", [S, E], F32, kind="ExternalInput")
    x_v = nc.dram_tensor("x_v", [S, E], vdt, kind="ExternalInput")
    wq = nc.dram_tensor("wq", [NPAIR, ET, 128, 128], F32, kind="ExternalInput")
    wk = nc.dram_tensor("wk", [NPAIR, ET, 128, 128], F32, kind="ExternalInput")
    wv = nc.dram_tensor("wv", [NPAIR, ET, 128, 128], vdt, kind="ExternalInput")
    bqs = nc.dram_tensor("bqs", [128, NPAIR], F32, kind="ExternalInput")
    bks = nc.dram_tensor("bks", [128, NPAIR], F32, kind="ExternalInput")
    bvb = nc.dram_tensor("bvb", [128, NPAIR, 128], F32, kind="ExternalInput")
    wo = nc.dram_tensor("wo", [NPAIR, 128, E], F32, kind="ExternalInput")
    out_p = nc.dram_tensor("out_p", [S, E], F32, kind="ExternalOutput")

    with tile.TileContext(nc) as tc:
        with (
            tc.tile_pool(name="const", bufs=1) as const_pool,
            tc.tile_pool(name="persist", bufs=1) as persist,
        ):
            ident = const_pool.tile([128, 128], F32, name="ident")
            masks.make_identity(nc, ident[:])
            ident_v = const_pool.tile([128, 128], vdt, name="ident_v")
            masks.make_identity(nc, ident_v[:])
            marg = const_pool.tile([128, 1], F32, name="marg")
            nc.gpsimd.memset(marg[:], -MARGIN)
            # all-ones tile for the rank-1 (-m) accumulate
            ones_st = const_pool.tile([128, 128], F32, name="ones_st")
            nc.gpsimd.memset(ones_st[:], 1.0)
            ones_r = const_pool.tile([128, 128], BF16, name="ones_r")
            nc.vector.tensor_copy(ones_r[:], ones_st[:])

            bqs_sb = const_pool.tile([128, NPAIR], F32, name="bqs")
            nc.sync.dma_start(bqs_sb[:], bqs[:])
            bks_sb = const_pool.tile([128, NPAIR], F32, name="bks")
            nc.sync.dma_start(bks_sb[:], bks[:])
            bvb_sb = const_pool.tile([128, NPAIR, 128], F32, name="bvb")
            nc.sync.dma_start(bvb_sb[:], bvb[:])
            # out-projection weights, cast to bf16 on device
            wo_st = const_pool.tile([128, NPAIR, E], F32, name="wo_st")
            nc.sync.dma_start(wo_st[:], wo.rearrange("c p e -> p c e"))
            wo_sb = const_pool.tile([128, NPAIR, E], BF16, name="wo")
            nc.vector.tensor_copy(wo_sb[:], wo_st[:])

            # persistent per-pair packed tensors (rows 0:64 = even head,
            # rows 64:128 = odd head of the pair)
            qr = [persist.tile([128, S], BF16, name=f"qr{p}") for p in range(NPAIR)]
            ql = [persist.tile([128, S], BF16, name=f"ql{p}") for p in range(NPAIR)]
            kr = [persist.tile([128, S], BF16, name=f"kr{p}") for p in range(NPAIR)]
            kl = [persist.tile([128, S], BF16, name=f"kl{p}") for p in range(NPAIR)]
            # -(rowmax)-MARGIN per pair: even head on partition 0, odd head
            # on partition 64 so the two rank-1 (-m) matmuls pair in the
            # array's row groups
            nm_p = [persist.tile([128, S], BF16, name=f"nmp{p}")
                    for p in range(NPAIR)]
            vh = [persist.tile([128, JT, 65], vdt, name=f"vh{h}")
                  for h in range(HPC)]
            ctxT = [persist.tile([128, S], BF16, name=f"ctxT{c}") for c in range(2)]

            for h in range(HPC):
                nc.gpsimd.memset(vh[h][:, :, 64:65], 1.0)

            # ---- phase 1: load, transpose, project ----
            with (
                tc.tile_pool(name="stage", bufs=3) as stage_pool,
                tc.tile_pool(name="xt", bufs=3) as xt_pool,
                tc.tile_pool(name="wght", bufs=1) as w_pool,
                tc.tile_pool(name="scr", bufs=3) as scr_pool,
                tc.tile_pool(name="ps_tin", bufs=3, space="PSUM") as ps_tin,
                tc.tile_pool(name="ps_proj", bufs=4, space="PSUM") as ps_proj,
            ):
                for x_dram, w_dram, which in ((x_q, wq, "q"), (x_k, wk, "k"),
                                              (x_v, wv, "v")):
                    dt_in = vdt if which == "v" else F32
                    id_in = ident_v if which == "v" else ident
                    w_sb = w_pool.tile([128, NPAIR, ET, 128], dt_in, name="w_in")
                    nc.sync.dma_start(w_sb[:], w_dram.rearrange("p t e d -> e p t d"))
                    for st in range(ST):
                        sl = bass.ts(st, 128)
                        stage = stage_pool.tile([128, E], dt_in, name="stage")
                        nc.sync.dma_start(stage[:], x_dram[sl, :])
                        xt = xt_pool.tile([128, ET, 128], dt_in, name="xt")
                        for et in range(ET):
                            pst = ps_tin.tile([128, 128], dt_in, name="pst")
                            nc.tensor.transpose(pst[:], stage[:, bass.ts(et, 128)],
                                                id_in[:])
                            nc.vector.tensor_copy(xt[:, et, :], pst[:])
                        for p in range(NPAIR):
                            psp = ps_proj.tile([128, 128], F32, name="psp")
                            for et in range(ET):
                                if which == "v":
                                    lhsT, rhs = xt[:, et, :], w_sb[:, p, et, :]
                                else:
                                    lhsT, rhs = w_sb[:, p, et, :], xt[:, et, :]
                                nc.tensor.matmul(psp[:], lhsT, rhs,
                                                 start=(et == 0), stop=(et == ET - 1))
                            if which == "q":
                                qex = scr_pool.tile([128, 128], F32, name="qex")
                                nc.scalar.activation(qex[:], psp[:], AF.Identity,
                                                     bias=bqs_sb[:, p:p + 1],
                                                     scale=0.125)
                                nc.vector.tensor_copy(qr[p][:, sl], qex[:])
                                nc.vector.scalar_tensor_tensor(
                                    out=ql[p][:, sl], in0=qex[:], scalar=1.0,
                                    in1=qr[p][:, sl],
                                    op0=ALU.mult, op1=ALU.subtract)
                            elif which == "k":
                                kex = scr_pool.tile([128, 128], F32, name="qex")
                                nc.scalar.activation(kex[:], psp[:], AF.Identity,
                                                     bias=bks_sb[:, p:p + 1],
                                                     scale=1.0)
                                nc.vector.tensor_copy(kr[p][:, sl], kex[:])
                                nc.vector.scalar_tensor_tensor(
                                    out=kl[p][:, sl], in0=kex[:], scalar=1.0,
                                    in1=kr[p][:, sl],
                                    op0=ALU.mult, op1=ALU.subtract)
                            else:
                                for hp in range(2):
                                    h = p * 2 + hp
                                    dsl = bass.ts(hp, 64)
                                    nc.vector.scalar_tensor_tensor(
                                        out=vh[h][:, st, 0:64],
                                        in0=psp[:, dsl], scalar=1.0,
                                        in1=bvb_sb[:, p, dsl],
                                        op0=ALU.mult, op1=ALU.add)

            # ---- phase 2+3: attention + out-projection ----
            with (
                tc.tile_pool(name="small", bufs=8) as small,
                tc.tile_pool(name="attw", bufs=1) as att_pool,
                tc.tile_pool(name="ctxn", bufs=4) as ctx_pool,
                tc.tile_pool(name="ps_a", bufs=2, space="PSUM") as ps_a,
                tc.tile_pool(name="ps_b", bufs=4, space="PSUM") as ps_b,
                tc.tile_pool(name="ps_m", bufs=2, space="PSUM") as ps_m,
            ):
                for ib in range(IB):
                    isl = bass.ts(ib, 512)
                    for p in range(NPAIR):
                        rows = [bass.ts(0, 64), bass.ts(1, 64)]
                        # A: approximate row max of logits (hi parts only);
                        # the two heads' 64-row matmuls pair in the array
                        for it in range(4):
                            i0 = ib * 512 + it * 128
                            itsl = bass.ds(i0, 128)
                            nm2 = [small.tile([128, 4], F32, name="nm2")
                                   for hp in range(2)]
                            for jh in range(4):
                                jsl = bass.ts(jh, 512)
                                psa = [ps_a.tile([128, 512], F32, name="psa")
                                       for hp in range(2)]
                                for hp in range(2):
                                    nc.tensor.matmul(
                                        psa[hp][:], qr[p][rows[hp], itsl],
                                        kr[p][rows[hp], jsl],
                                        start=True, stop=True)
                                for hp in range(2):
                                    nc.vector.reduce_max(
                                        nm2[hp][:, jh:jh + 1], psa[hp][:],
                                        axis=AX.X)
                            for hp in range(2):
                                nm = small.tile([128, 1], F32, name="nm")
                                nc.vector.reduce_max(nm[:], nm2[hp][:], axis=AX.X,
                                                     negate=True)
                                pst2 = ps_m.tile([128, 128], F32, name="pst2")
                                nc.tensor.transpose(pst2[0:1, :], nm[:],
                                                    ident[:])
                                if hp == 0:
                                    nc.scalar.activation(
                                        nm_p[p][0:1, itsl], pst2[0:1, :],
                                        AF.Identity, bias=marg[0:1, 0:1], scale=1.0)
                                else:
                                    # engines cannot shift partitions; bounce
                                    # through a scratch row + tiny SBUF DMA
                                    nms = small.tile([1, 128], BF16, name="nms")
                                    nc.scalar.activation(
                                        nms[:], pst2[0:1, :],
                                        AF.Identity, bias=marg[0:1, 0:1], scale=1.0)
                                    nc.sync.dma_start(nm_p[p][64:65, itsl], nms[:])
                        # B: shifted scores, 3-term f32r hi/lo, head-paired;
                        # -m applied via paired rank-1 accumulate; then exp
                        attT = [att_pool.tile([128, JT, 512], vdt, name=f"attT{hp}")
                                for hp in range(2)]
                        for jt in range(JT):
                            jsl = bass.ts(jt, 128)
                            psb = [ps_b.tile([128, 512], F32, name="psb")
                                   for hp in range(2)]
                            for hp in range(2):
                                nc.tensor.matmul(psb[hp][:], kr[p][rows[hp], jsl],
                                                 qr[p][rows[hp], isl],
                                                 start=True, stop=False)
                            for hp in range(2):
                                nc.tensor.matmul(psb[hp][:], kr[p][rows[hp], jsl],
                                                 ql[p][rows[hp], isl],
                                                 start=False, stop=False)
                            for hp in range(2):
                                nc.tensor.matmul(psb[hp][:], kl[p][rows[hp], jsl],
                                                 qr[p][rows[hp], isl],
                                                 start=False, stop=False)
                            for hp in range(2):
                                r0 = hp * 64
                                nc.tensor.matmul(psb[hp][:],
                                                 ones_r[r0:r0 + 1, 0:128],
                                                 nm_p[p][r0:r0 + 1, isl],
                                                 start=False, stop=True)
                            for hp in range(2):
                                nc.scalar.activation(attT[hp][:, jt, :],
                                                     psb[hp][:], AF.Exp)
                        # C: att @ V-hat, normalize, transpose ctx
                        for hp in range(2):
                            h = p * 2 + hp
                            for it in range(4):
                                i0 = ib * 512 + it * 128
                                psc = ps_m.tile([128, 128], F32, name="pst2")
                                for jt in range(JT):
                                    nc.tensor.matmul(
                                        psc[:, 0:65],
                                        attT[hp][:, jt, bass.ts(it, 128)],
                                        vh[h][:, jt, :],
                                        start=(jt == 0), stop=(jt == JT - 1))
                                recip = small.tile([128, 1], F32, name="recip")
                                nc.vector.reciprocal(recip[:], psc[:, 64:65])
                                ctxn = ctx_pool.tile([128, 64], BF16, name="ctxn")
                                nc.vector.tensor_scalar_mul(ctxn[:], psc[:, 0:64],
                                                            recip[:])
                                pst2 = ps_m.tile([128, 128], BF16, name="pst2")
                                nc.tensor.transpose(pst2[0:64, :], ctxn[:],
                                                    ident_v[:])
                                nc.vector.tensor_copy(
                                    ctxT[h // 2][bass.ts(h % 2, 64),
                                                 bass.ds(i0, 128)],
                                    pst2[0:64, :])
                    # out-projection for this i-block (float32r)
                    for it in range(4):
                        i0 = ib * 512 + it * 128
                        for eh in range(2):
                            pso = ps_b.tile([128, 512], F32, name="psb")
                            for ct in range(2):
                                nc.tensor.matmul(pso[:],
                                                 ctxT[ct][:, bass.ds(i0, 128)],
                                                 wo_sb[:, ct, bass.ts(eh, 512)],
                                                 start=(ct == 0), stop=(ct == 1))
                            outsb = ctx_pool.tile([128, 512], F32, name="outsb")
                            nc.scalar.copy(outsb[:], pso[:])
                            nc.sync.dma_start(out_p[bass.ds(i0, 128),
                                                    bass.ts(eh, 512)], outsb[:])
    nc.finalize()
    return nc


_NC_CACHE = None


def _get_nc():
    global _NC_CACHE
    if _NC_CACHE is None:
        _NC_CACHE = build_bass()
    return _NC_CACHE


def _prep_core_inputs(inputs, core):
    b, hg = core // 4, core % 4
    h0 = hg * HPC
    q, k, v = inputs["q"], inputs["k"], inputs["v"]
    Wq, Wk, Wv = inputs["Wq"], inputs["Wk"], inputs["Wv"]
    bq, bk, bv = inputs["bq"], inputs["bk"], inputs["bv"]
    Wo = inputs["Wo"]

    def pack_w(W):
        # [NPAIR, ET, 128, 128]: pair p, e-tile t -> [W[h0+2p] | W[h0+2p+1]]
        out = np.empty((NPAIR, ET, 128, 128), np.float32)
        for p in range(NPAIR):
            pair = np.concatenate([W[h0 + 2 * p], W[h0 + 2 * p + 1]], axis=1)
            out[p] = pair.reshape(ET, 128, 128)
        return out

    def pack_bcol(bias, scale):
        out = np.empty((128, NPAIR), np.float32)
        for p in range(NPAIR):
            out[:, p] = np.concatenate(
                [bias[h0 + 2 * p], bias[h0 + 2 * p + 1]]) * scale
        return out

    bvb = np.empty((128, NPAIR, 128), np.float32)
    for p in range(NPAIR):
        bvb[:, p, :] = np.concatenate([bv[h0 + 2 * p], bv[h0 + 2 * p + 1]])[None, :]

    wo_rows = Wo[h0 * DH:(h0 + HPC) * DH, :]  # [256, E]
    vdt_np = mybir.dt.np(ATT_DTYPE)
    return {
        "x_q": np.ascontiguousarray(q[b]),
        "x_k": np.ascontiguousarray(k[b]),
        "x_v": np.ascontiguousarray(v[b]).astype(vdt_np),
        "wq": pack_w(Wq), "wk": pack_w(Wk),
        "wv": pack_w(Wv).astype(vdt_np),
        "bqs": pack_bcol(bq, 0.125), "bks": pack_bcol(bk, 1.0), "bvb": bvb,
        "wo": np.ascontiguousarray(wo_rows.reshape(NPAIR, 128, E)),
    }


def run(inputs, trace=False, **kw):
    inputs = {k: np.asarray(v) for k, v in inputs.items()}
    nc = _get_nc()
    in_maps = [_prep_core_inputs(inputs, c) for c in range(NCORES)]
    res = run_bass_kernel_spmd(nc, in_maps, list(range(NCORES)), trace=trace, **kw)
    bo = inputs["bo"]
    out = np.empty((B, S, E), np.float32)
    for b in range(B):
        acc = res.results[b * 4]["out_p"].astype(np.float32)
        for c in range(b * 4 + 1, b * 4 + 4):
            acc = acc + res.results[c]["out_p"]
        out[b] = acc + bo[None, :]
    return out, res


def kernel(**inputs):
    out, _ = run(inputs)
    return out
